# revision 11
# baseline (speedup 1.0000x reference)
"""Causal self-attention (B=2, L=2048, D=1024, H=16) on 8 trn2 NeuronCores.

Sharding: core c = 4*b + g handles batch b and head group g (4 heads).
Per core: QKV projection for its heads' weight columns (tensor-parallel),
flash-style causal attention for its 4 heads, and a partial output
projection over its 256 head-dims (row-parallel).  The host sums the 4
partial projections per batch and adds bproj.

Device layout: activations kept transposed (feature-major) throughout:
  xT [D, L] (bf16, DMA'd as contiguous [128,512] tiles) -> Q^T packed per
  head-pair [128, L]; K^T stored as 4 zero-padded [128, L] bf16 tensors
  (head h occupies its 64 partition rows, the rest are zero) so score
  matmuls run at K=128 contraction; V in natural [L, 4*65] bf16 layout --
  col 64 of each head group is 1.0 so the attention row-sum Z rides along
  in the AV matmul; S^T tiles [k, q] so softmax needs no transposes; both
  heads of a pair share one [128,1024] two-bank psum tile so off-diagonal
  exp runs as a single ACT op; causal diagonal handled by a 0/1 bf16
  triangular MULTIPLY on the exp output (GpSimd, SBUF-only engine);
  softmax 1/Z computed once per block on a [2,512] tile (both heads) and
  broadcast through a selector matmul.  All matmuls bf16, fp32 psum.
Input/output DMA uses tile-contiguous DRAM layouts and is issued
round-robin across engine queues (the Sync queue alone serializes at
~0.6us per descriptor).
"""

import sys
import types

import numpy as np


def _install_ntff_shim():
    """The container's antenv stub lacks axon_hooks; recreate it so
    run_bass_kernel_spmd(trace=True) can reach the NTFF profiler."""
    if "antenv.axon_hooks" in sys.modules:
        return
    try:
        import antenv
        from trn_agent_boot.trn_boot import _ntff_profile_via_ctypes
    except Exception:
        return
    mod = types.ModuleType("antenv.axon_hooks")
    hook = _ntff_profile_via_ctypes("/opt/axon/libaxon_pjrt.so")
    mod.get_axon_ntff_profile_hook = lambda: hook
    mod.set_axon_ntff_profile_hook = lambda h: None
    sys.modules["antenv.axon_hooks"] = mod
    antenv.axon_hooks = mod


_install_ntff_shim()

import ml_dtypes  # noqa: E402

import concourse.bass as bass  # noqa: E402
import concourse.mybir as mybir  # noqa: E402
import concourse.tile as tile  # noqa: E402
from concourse.bass_utils import run_bass_kernel_spmd  # noqa: E402
from concourse.vector_clock import ScopedClock, VectorClock  # noqa: E402

B, L, D, H = 2, 2048, 1024, 16
HD = D // H  # 64
N_CORES = 8
HPC = 4  # heads per core
CD = HPC * HD  # 256 head-dims per core
VW = HPC * (HD + 1)  # 260 interleaved V columns (64 vals + ones col per head)
SCALE = HD**-0.5  # 0.125
F32 = mybir.dt.float32
B16 = mybir.dt.bfloat16
NPB16 = ml_dtypes.bfloat16

KT = L // 128  # 16 k-tiles of 128 keys
NS = L // 512  # 4 query chunks of 512
N_DK = D // 128  # 8 feature k-tiles
AV_DELAY = 6  # AV matmul issues this many (k,h)-steps behind its exp


class _TileContext(tile.TileContext):
    """Split exit-drain sem waits to 1 per drain; this walrus build's
    CTRL codegen rejects drains with 2+ sync waits."""

    def _drain_and_barrier(self, tick_clock, wait_clock):
        g = tick_clock.global_clock
        n = len(g)
        procs = [i for i in range(n) if g[i] > 0]
        for p in procs:
            vec = [g[i] if i == p else 0 for i in range(n)]
            d = self.nc.sync.drain()
            wait_clock.add_sem_waits(d.ins, ScopedClock({None: VectorClock(vec)}))
        self.nc.all_engine_barrier()
        popped = self.nc._tile_sem_poison_stack.pop()
        assert popped is self._sem_poison
        self.nc.clear_and_free_semaphores(list(self.sems.allocated().values()))
        self.nc.all_engine_barrier()


def _split_multi_waits(nc):
    """This walrus build's codegen accepts only ONE sync wait per
    instruction; hoist extra waits onto preceding same-engine NOPs."""
    for f in nc.m.functions:
        for blk in f.blocks:
            orig = list(blk.instructions)
            expanded = []
            changed = False
            for ins in orig:
                si = ins.sync_info
                if si is not None and si.on_wait is not None and len(si.on_wait) > 1:
                    changed = True
                    waits = list(si.on_wait)
                    eng = nc.engines[ins.engine]
                    for w in waits[:-1]:
                        nop = eng.nop(nofuse=True).ins
                        # eng.nop() auto-appends to the CURRENT bb; pull it
                        # out -- we re-insert it before `ins` in ins's bb.
                        nc.cur_bb.bb.instructions.remove(nop)
                        nop.sync_info = mybir.SyncInfo(on_wait=[w], on_update=[])
                        expanded.append(nop)
                    ins.sync_info = mybir.SyncInfo(
                        on_wait=[waits[-1]], on_update=list(si.on_update or [])
                    )
                expanded.append(ins)
            if changed:
                il = blk.instructions
                for ins in list(il):
                    il.remove(ins)
                for ins in expanded:
                    il.append(ins)


def _build_program():
    nc = bass.Bass()
    # tile-contiguous DRAM layouts: each DMA descriptor moves one
    # contiguous [128, C] block (large linear packets, no row striding)
    xT_d = nc.dram_tensor("xT", [N_DK * NS, 128, 512], B16, kind="ExternalInput").ap()
    wqkv_d = nc.dram_tensor(
        "wqkv", [N_DK, 128, 2 * CD + VW], B16, kind="ExternalInput"
    ).ap()
    bqk_d = nc.dram_tensor("bqk", [128, 4], F32, kind="ExternalInput").ap()
    bv_d = nc.dram_tensor("bv", [1, VW], B16, kind="ExternalInput").ap()
    wproj_d = nc.dram_tensor("wproj", [2, 128, D], B16, kind="ExternalInput").ap()
    bproj_d = nc.dram_tensor("bproj", [128, N_DK], F32, kind="ExternalInput").ap()
    onesr_d = nc.dram_tensor("onesr", [1, 512], B16, kind="ExternalInput").ap()
    tri_d = nc.dram_tensor("trimask", [128, 128], B16, kind="ExternalInput").ap()
    sel_d = nc.dram_tensor("sel", [2, 128], B16, kind="ExternalInput").ap()
    zer_d = nc.dram_tensor("zer", [64, L], B16, kind="ExternalInput").ap()
    yT_d = nc.dram_tensor("yT", [NS * N_DK, 128, 512], B16, kind="ExternalOutput").ap()

    mm = nc.tensor.matmul

    with _TileContext(nc) as tc, tc.tile_pool(name="sb", bufs=1) as sb, tc.tile_pool(
        name="ps", bufs=1, space="PSUM"
    ) as ps:
        # ---- constants (host-supplied; memset/affine_select of f32r
        # fail this walrus build's ISA checks) ----
        ones = sb.tile([1, 512], B16, tag="ones", bufs=1)
        nc.sync.dma_start(out=ones[:], in_=onesr_d[:])
        tri = sb.tile([128, 128], B16, tag="tri", bufs=1)
        nc.sync.dma_start(out=tri[:], in_=tri_d[:])
        sel = sb.tile([2, 128], B16, tag="sel", bufs=1)
        nc.sync.dma_start(out=sel[:], in_=sel_d[:])
        bqk = sb.tile([128, 4], F32, tag="bqk", bufs=1)
        nc.sync.dma_start(out=bqk[:], in_=bqk_d[:])
        bv = sb.tile([1, VW], B16, tag="bv", bufs=1)
        nc.sync.dma_start(out=bv[:], in_=bv_d[:])
        bproj = sb.tile([128, N_DK], F32, tag="bproj", bufs=1)
        nc.sync.dma_start(out=bproj[:], in_=bproj_d[:])

        # ---- persistent SBUF tensors; DMA issue spread across engine
        # queues (sync alone serializes at ~0.6us/descriptor) ----
        dma_engs = [nc.sync, nc.gpsimd, nc.scalar]
        _rr = [0]

        def dma(out, in_, engs=None):
            pool = engs if engs is not None else dma_engs
            e = pool[_rr[0] % len(pool)]
            _rr[0] += 1
            e.dma_start(out=out, in_=in_)

        wqkv = [
            sb.tile([128, 2 * CD + VW], B16, tag=f"wqkv{k}", bufs=1, name=f"wqkv{k}")
            for k in range(N_DK)
        ]
        xTc = [
            [
                sb.tile([128, 512], B16, tag=f"xT{k}_{s}", bufs=1, name=f"xT{k}_{s}")
                for s in range(NS)
            ]
            for k in range(N_DK)
        ]
        # first wave: everything the s=0 QKV accumulation chains need
        for k in range(N_DK):
            dma(wqkv[k][:], wqkv_d[k])
            dma(xTc[k][0][:], xT_d[k * NS + 0])
        late_engs = [nc.sync, nc.gpsimd, nc.scalar]
        for s in range(1, NS):
            for k in range(N_DK):
                dma(xTc[k][s][:], xT_d[k * NS + s], engs=late_engs)
        wproj = []
        for kt in range(2):
            t = sb.tile([128, D], B16, tag=f"wproj{kt}", bufs=1)
            dma(t[:], wproj_d[kt], engs=late_engs)
            wproj.append(t)
        # Q^T packed per head pair (rows 0-63 = head 2p, 64-127 = head 2p+1)
        qT = [sb.tile([128, L], B16, tag=f"qT{p}", bufs=1, name=f"qT{p}") for p in range(2)]
        # K^T zero-padded per head: kz[p][h] has head 2p+h in its own 64
        # rows, zeros elsewhere -> K=128 score matmuls pick out one head.
        kz = [
            [
                sb.tile([128, L], B16, tag=f"kz{p}{h}", bufs=1, name=f"kz{p}{h}")
                for h in range(2)
            ]
            for p in range(2)
        ]
        for p in range(2):
            dma(kz[p][0][64:128, :], zer_d[:], engs=late_engs)
            dma(kz[p][1][0:64, :], zer_d[:], engs=late_engs)
        # V natural layout, 16 token tiles of [128, 4*65]; col 64 of each
        # head group = 1.0 (from interleaved W zero-cols + bias ones row)
        vsb = [sb.tile([128, VW], B16, tag=f"v{t}", bufs=1, name=f"v{t}") for t in range(KT)]
        attnT = [sb.tile([128, L], B16, tag=f"attnT{k}", bufs=1, name=f"attnT{k}") for k in range(2)]

        # ================= QKV projection =================
        for s in range(NS):
            # Q/K part: out[wcol, token] = wqkv[:, m-tile].T @ xT
            for m in range(4):
                p_qk = ps.tile([128, 512], F32, tag="mm", bufs=2)
                for k in range(N_DK):
                    mm(
                        p_qk[:],
                        wqkv[k][:, 128 * m : 128 * (m + 1)],
                        xTc[k][s][:],
                        start=(k == 0),
                        stop=(k == N_DK - 1),
                    )
                # copy to SBUF (bf16) with per-partition (wcol) bias add
                cs = slice(512 * s, 512 * (s + 1))
                TSADD = mybir.AluOpType.add
                if m < 2:
                    nc.vector.tensor_scalar(
                        qT[m][:, cs], p_qk[:], bqk[:, m : m + 1], None, op0=TSADD
                    )
                else:
                    p = m - 2
                    nc.vector.tensor_scalar(
                        kz[p][0][0:64, cs], p_qk[0:64, :],
                        bqk[0:64, m : m + 1], None, op0=TSADD,
                    )
                    nc.vector.tensor_scalar(
                        kz[p][1][64:128, cs], p_qk[64:128, :],
                        bqk[64:128, m : m + 1], None, op0=TSADD,
                    )
            # V part: out[token, vcol] = xT[:, tt].T @ wv_interleaved
            for j in range(4):
                t = 4 * s + j
                p_v = ps.tile([128, VW], F32, tag="mm", bufs=2)
                for k in range(N_DK):
                    mm(
                        p_v[:],
                        xTc[k][s][:, 128 * j : 128 * (j + 1)],
                        wqkv[k][:, 2 * CD : 2 * CD + VW],
                        start=(k == 0),
                        stop=False,
                    )
                mm(p_v[:], ones[0:1, 0:128], bv[:], start=False, stop=True)
                nc.vector.tensor_copy(vsb[t][:], p_v[:])

        # ================= attention =================
        # Software-pipelined across (pair, s) blocks:
        #  - AV matmuls issue AV_DELAY steps behind their exp (cross-block)
        #  - block b-1's tail AVs + Z/unnormalized-copy flush after block
        #    b's first steps; block b-2's normalize follows
        # so the PE never sits on an exp/reciprocal dependency.
        def emit_recip(av):
            # block end: pull Z (bf16) and the unnormalized AV rows out of
            # psum; the division happens next block on the broadcast tile.
            rzs = []
            for h in range(2):
                z = sb.tile([1, 512], B16, tag="rz", bufs=4, name="z")
                nc.vector.tensor_copy(z[:], av[h][64:65, :])
                un = sb.tile([64, 512], F32, tag="un", bufs=4, name="un")
                nc.vector.tensor_copy(un[:], av[h][0:64, :])
                rzs.append((z, un))
            return rzs

        def emit_norm(pair, q0, rzs):
            for h in range(2):
                z, un = rzs[h]
                # broadcast Z_h to 64 partitions via ones matmul, then a
                # single DVE divide (no serial reciprocal op)
                bc_ps = ps.tile([64, 512], F32, tag="mm", bufs=2, name="bc_ps")
                mm(bc_ps[:], ones[0:1, 0:64], z[:], start=True, stop=True)
                bc = sb.tile([64, 512], F32, tag="bc_sb", bufs=2, name="bc")
                with nc.allow_low_precision(reason="bcast 1/Z"):
                    nc.vector.reciprocal(bc[:], bc_ps[:])
                if h == 0:
                    nc.vector.tensor_tensor(
                        attnT[pair][0:64, q0 : q0 + 512],
                        un[:],
                        bc[:],
                        op=mybir.AluOpType.mult,
                    )
                else:
                    tmp = sb.tile([64, 512], B16, tag="ntmp", bufs=2, name="tmp")
                    nc.vector.tensor_tensor(
                        tmp[:], un[:], bc[:], op=mybir.AluOpType.mult
                    )
                    nc.gpsimd.dma_start(
                        out=attnT[pair][64:128, q0 : q0 + 512], in_=tmp[:]
                    )

        pending = []  # (block_id, mm_args, mm_kwargs)
        fin_prev = None  # (block_id, pair, q0, av) awaiting tail-flush + recip
        norm_prev = None  # (pair, q0, rzs) awaiting normalize
        blocks = [(p, s) for p in range(2) for s in range(NS)]
        for bid, (pair, s) in enumerate(blocks):
            q0 = 512 * s
            n_k = 4 * s + 4
            av = [
                ps.tile([65, 512], F32, tag=f"av{h}", bufs=1, name=f"av{h}")
                for h in range(2)
            ]
            for k in range(n_k):
                k0 = 128 * k
                diag_t = k - 4 * s
                lo = 128 * diag_t if diag_t >= 0 else 0
                # both heads' scores into one 2-bank psum tile; halves at
                # column offsets 0 / 512 so the off-diagonal exp covers both
                s_ps = ps.tile([128, 1024], F32, tag="st", bufs=2)
                pt = sb.tile([128, 1024], B16, tag="pt", bufs=AV_DELAY // 2 + 2)
                for h in range(2):
                    c0 = 512 * h
                    mm(
                        s_ps[:, c0 + lo : c0 + 512],
                        kz[pair][h][:, k0 : k0 + 128],
                        qT[pair][:, q0 + lo : q0 + 512],
                        start=True,
                        stop=True,
                    )
                if diag_t >= 0:
                    for h in range(2):
                        c0 = 512 * h
                        nc.scalar.activation(
                            pt[:, c0 + lo : c0 + 512],
                            s_ps[:, c0 + lo : c0 + 512],
                            mybir.ActivationFunctionType.Exp,
                            scale=SCALE,
                        )
                    # causal mask: multiply the diagonal 128x128 block of
                    # exp(S^T) by a 0/1 lower-triangle (GpSimd, sbuf-only)
                    for h in range(2):
                        c0 = 512 * h
                        nc.gpsimd.tensor_tensor(
                            pt[:, c0 + lo : c0 + lo + 128],
                            pt[:, c0 + lo : c0 + lo + 128],
                            tri[:],
                            op=mybir.AluOpType.mult,
                        )
                else:
                    nc.scalar.activation(
                        pt[:],
                        s_ps[:],
                        mybir.ActivationFunctionType.Exp,
                        scale=SCALE,
                    )
                for h in range(2):
                    hg = 2 * pair + h
                    c0 = 512 * h
                    pending.append(
                        (
                            bid,
                            (
                                av[h][0:65, lo:512],
                                vsb[k][:, 65 * hg : 65 * hg + 65],
                                pt[:, c0 + lo : c0 + 512],
                            ),
                            dict(
                                start=(k == 0),
                                stop=(k == n_k - 1),
                                skip_group_check=True,
                            ),
                        )
                    )
                    while len(pending) > AV_DELAY:
                        _, a, kw = pending.pop(0)
                        mm(*a, **kw)
                if k == 1 and fin_prev is not None:
                    # flush the previous block's tail AVs, free its av psum
                    # via recip + unnormalized copy, then run the normalize
                    # of the block before that
                    pbid = fin_prev[0]
                    while pending and pending[0][0] == pbid:
                        _, a, kw = pending.pop(0)
                        mm(*a, **kw)
                    if norm_prev is not None:
                        emit_norm(*norm_prev)
                        norm_prev = None
                    _, ppair, pq0, pav = fin_prev
                    norm_prev = (ppair, pq0, emit_recip(pav))
                    fin_prev = None
            fin_prev = (bid, pair, q0, av)
        while pending:
            _, a, kw = pending.pop(0)
            mm(*a, **kw)
        if norm_prev is not None:
            emit_norm(*norm_prev)
        _, ppair, pq0, pav = fin_prev
        emit_norm(ppair, pq0, emit_recip(pav))

        # ================= output projection (partial) =================
        for s in range(NS):
            for m in range(N_DK):
                p_y = ps.tile([128, 512], F32, tag="mm", bufs=2)
                for kt in range(2):
                    mm(
                        p_y[:],
                        wproj[kt][:, 128 * m : 128 * (m + 1)],
                        attnT[kt][:, 512 * s : 512 * (s + 1)],
                        start=(kt == 0),
                        stop=(kt == 1),
                    )
                y_sb = sb.tile([128, 512], B16, tag="ysb", bufs=3)
                nc.scalar.activation(
                    y_sb[:], p_y[:], mybir.ActivationFunctionType.Identity,
                    bias=bproj[:, m : m + 1],
                )
                nc.gpsimd.dma_start(out=yT_d[s * N_DK + m], in_=y_sb[:])
    _split_multi_waits(nc)
    return nc


_NC_CACHE = None
LAST_RESULTS = None

_ONESR = np.ones((1, 512), dtype=NPB16)
_ZER = np.zeros((64, L), dtype=NPB16)
_I, _J = np.meshgrid(np.arange(128), np.arange(128), indexing="ij")
_TRI = (_J >= _I).astype(NPB16)  # 1.0 keep / 0.0 mask on the S^T diag block
_SEL = np.zeros((2, 128), dtype=NPB16)
_SEL[0, 0:64] = 1.0
_SEL[1, 64:128] = 1.0


def _make_in_maps(x, Wqkv, bqkv, Wproj, bproj):
    in_maps = []
    for c in range(N_CORES):
        b, g = divmod(c, 4)
        qc = slice(CD * g, CD * (g + 1))
        wq = Wqkv[:, qc]
        wk = Wqkv[:, D : 2 * D][:, qc]
        wv = Wqkv[:, 2 * D : 3 * D][:, qc]
        bq = bqkv[qc]
        bk = bqkv[D : 2 * D][qc]
        bvv = bqkv[2 * D : 3 * D][qc]
        # V columns interleaved per head: [wv_h (64 cols) | zeros col] so the
        # psum comes out in vsb layout; bv row gets [bv_h | 1.0].
        wv_i = np.zeros((D, VW), dtype=np.float32)
        bv_i = np.zeros((1, VW), dtype=np.float32)
        for h in range(HPC):
            wv_i[:, 65 * h : 65 * h + 64] = wv[:, 64 * h : 64 * h + 64]
            bv_i[0, 65 * h : 65 * h + 64] = bvv[64 * h : 64 * h + 64]
            bv_i[0, 65 * h + 64] = 1.0
        bqk_cols = np.concatenate([bq, bk]).reshape(4, 128).T  # [128, 4]
        xT = x[b].T.astype(NPB16)  # [D, L]
        xT_t = np.ascontiguousarray(
            xT.reshape(N_DK, 128, NS, 512).transpose(0, 2, 1, 3)
        ).reshape(N_DK * NS, 128, 512)
        wqkv_full = np.concatenate([wq, wk, wv_i], axis=1).astype(NPB16)
        wqkv_t = np.ascontiguousarray(wqkv_full.reshape(N_DK, 128, 2 * CD + VW))
        wproj_t = np.ascontiguousarray(
            Wproj[CD * g : CD * (g + 1), :].astype(NPB16).reshape(2, 128, D)
        )
        in_maps.append(
            {
                "xT": xT_t,
                "wqkv": wqkv_t,
                "bqk": np.ascontiguousarray(bqk_cols),
                "bv": bv_i.astype(NPB16),
                "wproj": wproj_t,
                "bproj": np.ascontiguousarray(
                    (bproj if g == 0 else np.zeros_like(bproj)).reshape(N_DK, 128).T
                ),
                "onesr": _ONESR,
                "trimask": _TRI,
                "sel": _SEL,
                "zer": _ZER,
            }
        )

    return in_maps


def kernel(x, Wqkv, bqkv, Wproj, bproj):
    global _NC_CACHE, LAST_RESULTS
    x = np.asarray(x, dtype=np.float32)
    Wqkv = np.asarray(Wqkv, dtype=np.float32)
    bqkv = np.asarray(bqkv, dtype=np.float32)
    Wproj = np.asarray(Wproj, dtype=np.float32)
    bproj = np.asarray(bproj, dtype=np.float32)

    if _NC_CACHE is None:
        _NC_CACHE = _build_program()
    nc = _NC_CACHE

    in_maps = _make_in_maps(x, Wqkv, bqkv, Wproj, bproj)
    res = run_bass_kernel_spmd(nc, in_maps, core_ids=list(range(N_CORES)))
    LAST_RESULTS = res

    out = np.empty((B, L, D), dtype=np.float32)
    for b in range(B):
        acc = res.results[4 * b]["yT"].astype(np.float32)
        for g in range(1, 4):
            acc = acc + res.results[4 * b + g]["yT"]
        # yT tiles indexed s*N_DK+m, each [128 dcols, 512 tokens]
        yT = acc.reshape(NS, N_DK, 128, 512).transpose(1, 2, 0, 3).reshape(D, L)
        out[b] = yT.T
    return out


# revision 14
# speedup vs baseline: 1.0523x; 1.0523x over previous
"""Causal self-attention (B=2, L=2048, D=1024, H=16) on 8 trn2 NeuronCores.

Sharding: core c = 4*b + g handles batch b and head group g (4 heads).
Per core: QKV projection for its heads' weight columns (tensor-parallel),
flash-style causal attention for its 4 heads, and a partial output
projection over its 256 head-dims (row-parallel).  The host sums the 4
partial projections per batch and adds bproj.

Device layout: activations kept transposed (feature-major) throughout:
  xT [D, L] (bf16, DMA'd as contiguous [128,512] tiles) -> Q^T packed per
  head-pair [128, L]; K^T stored as 4 zero-padded [128, L] bf16 tensors
  (head h occupies its 64 partition rows, the rest are zero) so score
  matmuls run at K=128 contraction; V in natural [L, 4*65] bf16 layout --
  col 64 of each head group is 1.0 so the attention row-sum Z rides along
  in the AV matmul; S^T tiles [k, q] so softmax needs no transposes; both
  heads of a pair share one [128,1024] two-bank psum tile so off-diagonal
  exp runs as a single ACT op; causal diagonal handled by a 0/1 bf16
  triangular MULTIPLY on the exp output (GpSimd, SBUF-only engine);
  softmax 1/Z computed once per block on a [2,512] tile (both heads) and
  broadcast through a selector matmul.  All matmuls bf16, fp32 psum.
Input/output DMA uses tile-contiguous DRAM layouts and is issued
round-robin across engine queues (the Sync queue alone serializes at
~0.6us per descriptor).
"""

import sys
import types

import numpy as np


def _install_ntff_shim():
    """The container's antenv stub lacks axon_hooks; recreate it so
    run_bass_kernel_spmd(trace=True) can reach the NTFF profiler."""
    if "antenv.axon_hooks" in sys.modules:
        return
    try:
        import antenv
        from trn_agent_boot.trn_boot import _ntff_profile_via_ctypes
    except Exception:
        return
    mod = types.ModuleType("antenv.axon_hooks")
    hook = _ntff_profile_via_ctypes("/opt/axon/libaxon_pjrt.so")
    mod.get_axon_ntff_profile_hook = lambda: hook
    mod.set_axon_ntff_profile_hook = lambda h: None
    sys.modules["antenv.axon_hooks"] = mod
    antenv.axon_hooks = mod


_install_ntff_shim()

import ml_dtypes  # noqa: E402

import concourse.bass as bass  # noqa: E402
import concourse.mybir as mybir  # noqa: E402
import concourse.tile as tile  # noqa: E402
from concourse.bass_utils import run_bass_kernel_spmd  # noqa: E402
from concourse.vector_clock import ScopedClock, VectorClock  # noqa: E402

B, L, D, H = 2, 2048, 1024, 16
HD = D // H  # 64
N_CORES = 8
HPC = 4  # heads per core
CD = HPC * HD  # 256 head-dims per core
VW = HPC * (HD + 1)  # 260 interleaved V columns (64 vals + ones col per head)
SCALE = HD**-0.5  # 0.125
F32 = mybir.dt.float32
B16 = mybir.dt.bfloat16
NPB16 = ml_dtypes.bfloat16

KT = L // 128  # 16 k-tiles of 128 keys
NS = L // 512  # 4 query chunks of 512
N_DK = D // 128  # 8 feature k-tiles
AV_DELAY = 6  # AV matmul issues this many (k,h)-steps behind its exp


class _TileContext(tile.TileContext):
    """Split exit-drain sem waits to 1 per drain; this walrus build's
    CTRL codegen rejects drains with 2+ sync waits."""

    def _drain_and_barrier(self, tick_clock, wait_clock):
        g = tick_clock.global_clock
        n = len(g)
        procs = [i for i in range(n) if g[i] > 0]
        for p in procs:
            vec = [g[i] if i == p else 0 for i in range(n)]
            d = self.nc.sync.drain()
            wait_clock.add_sem_waits(d.ins, ScopedClock({None: VectorClock(vec)}))
        self.nc.all_engine_barrier()
        popped = self.nc._tile_sem_poison_stack.pop()
        assert popped is self._sem_poison
        self.nc.clear_and_free_semaphores(list(self.sems.allocated().values()))
        self.nc.all_engine_barrier()


def _split_multi_waits(nc):
    """This walrus build's codegen accepts only ONE sync wait per
    instruction; hoist extra waits onto preceding same-engine NOPs."""
    for f in nc.m.functions:
        for blk in f.blocks:
            orig = list(blk.instructions)
            expanded = []
            changed = False
            for ins in orig:
                si = ins.sync_info
                if si is not None and si.on_wait is not None and len(si.on_wait) > 1:
                    changed = True
                    waits = list(si.on_wait)
                    eng = nc.engines[ins.engine]
                    for w in waits[:-1]:
                        nop = eng.nop(nofuse=True).ins
                        # eng.nop() auto-appends to the CURRENT bb; pull it
                        # out -- we re-insert it before `ins` in ins's bb.
                        nc.cur_bb.bb.instructions.remove(nop)
                        nop.sync_info = mybir.SyncInfo(on_wait=[w], on_update=[])
                        expanded.append(nop)
                    ins.sync_info = mybir.SyncInfo(
                        on_wait=[waits[-1]], on_update=list(si.on_update or [])
                    )
                expanded.append(ins)
            if changed:
                il = blk.instructions
                for ins in list(il):
                    il.remove(ins)
                for ins in expanded:
                    il.append(ins)


def _build_program():
    nc = bass.Bass()
    # tile-contiguous DRAM layouts: each DMA descriptor moves one
    # contiguous [128, C] block (large linear packets, no row striding)
    xT_d = nc.dram_tensor("xT", [N_DK * NS, 128, 512], B16, kind="ExternalInput").ap()
    wqkv_d = nc.dram_tensor(
        "wqkv", [N_DK, 128, 2 * CD + VW], B16, kind="ExternalInput"
    ).ap()
    bqk_d = nc.dram_tensor("bqk", [128, 4], F32, kind="ExternalInput").ap()
    bv_d = nc.dram_tensor("bv", [1, VW], B16, kind="ExternalInput").ap()
    wproj_d = nc.dram_tensor("wproj", [2, 128, D], B16, kind="ExternalInput").ap()
    bproj_d = nc.dram_tensor("bproj", [128, N_DK], F32, kind="ExternalInput").ap()
    onesr_d = nc.dram_tensor("onesr", [1, 512], B16, kind="ExternalInput").ap()
    tri_d = nc.dram_tensor("trimask", [128, 128], B16, kind="ExternalInput").ap()
    sel_d = nc.dram_tensor("sel", [2, 128], B16, kind="ExternalInput").ap()
    zer_d = nc.dram_tensor("zer", [64, L], B16, kind="ExternalInput").ap()
    yT_d = nc.dram_tensor("yT", [NS * N_DK, 128, 512], B16, kind="ExternalOutput").ap()

    mm = nc.tensor.matmul

    with _TileContext(nc) as tc, tc.tile_pool(name="sb", bufs=1) as sb, tc.tile_pool(
        name="ps", bufs=1, space="PSUM"
    ) as ps:
        # ---- constants (host-supplied; memset/affine_select of f32r
        # fail this walrus build's ISA checks) ----
        ones = sb.tile([1, 512], B16, tag="ones", bufs=1)
        nc.sync.dma_start(out=ones[:], in_=onesr_d[:])
        tri = sb.tile([128, 128], B16, tag="tri", bufs=1)
        nc.sync.dma_start(out=tri[:], in_=tri_d[:])
        sel = sb.tile([2, 128], B16, tag="sel", bufs=1)
        nc.sync.dma_start(out=sel[:], in_=sel_d[:])
        bqk = sb.tile([128, 4], F32, tag="bqk", bufs=1)
        nc.sync.dma_start(out=bqk[:], in_=bqk_d[:])
        bv = sb.tile([1, VW], B16, tag="bv", bufs=1)
        nc.sync.dma_start(out=bv[:], in_=bv_d[:])
        bproj = sb.tile([128, N_DK], F32, tag="bproj", bufs=1)
        nc.sync.dma_start(out=bproj[:], in_=bproj_d[:])

        # ---- persistent SBUF tensors; DMA issue spread across engine
        # queues (sync alone serializes at ~0.6us/descriptor) ----
        dma_engs = [nc.sync, nc.gpsimd, nc.scalar]
        _rr = [0]

        def dma(out, in_, engs=None):
            pool = engs if engs is not None else dma_engs
            e = pool[_rr[0] % len(pool)]
            _rr[0] += 1
            e.dma_start(out=out, in_=in_)

        wqkv = [
            sb.tile([128, 2 * CD + VW], B16, tag=f"wqkv{k}", bufs=1, name=f"wqkv{k}")
            for k in range(N_DK)
        ]
        xTc = [
            [
                sb.tile([128, 512], B16, tag=f"xT{k}_{s}", bufs=1, name=f"xT{k}_{s}")
                for s in range(NS)
            ]
            for k in range(N_DK)
        ]
        # first wave: everything the s=0 QKV accumulation chains need
        for k in range(N_DK):
            dma(wqkv[k][:], wqkv_d[k])
            dma(xTc[k][0][:], xT_d[k * NS + 0])
        late_engs = [nc.sync, nc.gpsimd, nc.scalar]
        for s in range(1, NS):
            for k in range(N_DK):
                dma(xTc[k][s][:], xT_d[k * NS + s], engs=late_engs)
        wproj = []
        for kt in range(2):
            t = sb.tile([128, D], B16, tag=f"wproj{kt}", bufs=1)
            dma(t[:], wproj_d[kt], engs=late_engs)
            wproj.append(t)
        # Q^T packed per head pair (rows 0-63 = head 2p, 64-127 = head 2p+1)
        qT = [sb.tile([128, L], B16, tag=f"qT{p}", bufs=1, name=f"qT{p}") for p in range(2)]
        # K^T zero-padded per head: kz[p][h] has head 2p+h in its own 64
        # rows, zeros elsewhere -> K=128 score matmuls pick out one head.
        kz = [
            [
                sb.tile([128, L], B16, tag=f"kz{p}{h}", bufs=1, name=f"kz{p}{h}")
                for h in range(2)
            ]
            for p in range(2)
        ]
        for p in range(2):
            dma(kz[p][0][64:128, :], zer_d[:], engs=late_engs)
            dma(kz[p][1][0:64, :], zer_d[:], engs=late_engs)
        # V natural layout, 16 token tiles of [128, 4*65]; col 64 of each
        # head group = 1.0 (from interleaved W zero-cols + bias ones row)
        vsb = [sb.tile([128, VW], B16, tag=f"v{t}", bufs=1, name=f"v{t}") for t in range(KT)]
        attnT = [sb.tile([128, L], B16, tag=f"attnT{k}", bufs=1, name=f"attnT{k}") for k in range(2)]

        # ============== interleaved QKV / attention / proj ==============
        # Emission order per s-chunk: QKV chunk s, then proj chunk s-2,
        # then attention blocks (0,s),(1,s).  Tile deps keep it correct;
        # interleaving keeps the PE queue full (no p-state resets) and
        # spreads the ACT exp work across the whole kernel.
        TSADD = mybir.AluOpType.add

        def emit_qkv_chunk(s):
            # Q/K part: out[wcol, token] = wqkv[:, m-tile].T @ xT
            for m in range(4):
                p_qk = ps.tile([128, 512], F32, tag="mm", bufs=2)
                for k in range(N_DK):
                    mm(
                        p_qk[:],
                        wqkv[k][:, 128 * m : 128 * (m + 1)],
                        xTc[k][s][:],
                        start=(k == 0),
                        stop=(k == N_DK - 1),
                    )
                # copy to SBUF (bf16) with per-partition (wcol) bias add
                cs = slice(512 * s, 512 * (s + 1))
                if m < 2:
                    nc.vector.tensor_scalar(
                        qT[m][:, cs], p_qk[:], bqk[:, m : m + 1], None, op0=TSADD
                    )
                else:
                    p = m - 2
                    nc.vector.tensor_scalar(
                        kz[p][0][0:64, cs], p_qk[0:64, :],
                        bqk[0:64, m : m + 1], None, op0=TSADD,
                    )
                    nc.vector.tensor_scalar(
                        kz[p][1][64:128, cs], p_qk[64:128, :],
                        bqk[64:128, m : m + 1], None, op0=TSADD,
                    )
            # V part: out[token, vcol] = xT[:, tt].T @ wv_interleaved
            for j in range(4):
                t = 4 * s + j
                p_v = ps.tile([128, VW], F32, tag="mm", bufs=2)
                for k in range(N_DK):
                    mm(
                        p_v[:],
                        xTc[k][s][:, 128 * j : 128 * (j + 1)],
                        wqkv[k][:, 2 * CD : 2 * CD + VW],
                        start=(k == 0),
                        stop=False,
                    )
                mm(p_v[:], ones[0:1, 0:128], bv[:], start=False, stop=True)
                nc.scalar.copy(vsb[t][:], p_v[:])

        def emit_recip(av):
            # block end: pull Z and the unnormalized AV rows out of psum
            rzs = []
            for h in range(2):
                z = sb.tile([1, 512], B16, tag="rz", bufs=4, name="z")
                nc.vector.tensor_copy(z[:], av[h][64:65, :])
                un = sb.tile([64, 512], F32, tag="un", bufs=4, name="un")
                nc.vector.tensor_copy(un[:], av[h][0:64, :])
                rzs.append((z, un))
            return rzs

        def emit_norm(pair, q0, rzs):
            # normalize on GpSimd (idle engine): broadcast Z along
            # partitions, then a single elementwise divide
            for h in range(2):
                z, un = rzs[h]
                bc_ps = ps.tile([64, 512], F32, tag="mm", bufs=2, name="bc_ps")
                mm(bc_ps[:], ones[0:1, 0:64], z[:], start=True, stop=True)
                bc = sb.tile([64, 512], F32, tag="bc_sb", bufs=2, name="bc")
                with nc.allow_low_precision(reason="bcast 1/Z"):
                    nc.vector.reciprocal(bc[:], bc_ps[:])
                # the normalize multiply runs on GpSimd (sbuf-only, idle)
                if h == 0:
                    nc.gpsimd.tensor_tensor(
                        attnT[pair][0:64, q0 : q0 + 512],
                        un[:],
                        bc[:],
                        op=mybir.AluOpType.mult,
                    )
                else:
                    tmp = sb.tile([64, 512], B16, tag="ntmp", bufs=2, name="tmp")
                    nc.gpsimd.tensor_tensor(
                        tmp[:], un[:], bc[:], op=mybir.AluOpType.mult
                    )
                    nc.gpsimd.dma_start(
                        out=attnT[pair][64:128, q0 : q0 + 512], in_=tmp[:]
                    )

        def emit_proj_chunk(u):
            for m in range(N_DK):
                p_y = ps.tile([128, 512], F32, tag="mm", bufs=2)
                for kt in range(2):
                    mm(
                        p_y[:],
                        wproj[kt][:, 128 * m : 128 * (m + 1)],
                        attnT[kt][:, 512 * u : 512 * (u + 1)],
                        start=(kt == 0),
                        stop=(kt == 1),
                    )
                y_sb = sb.tile([128, 512], B16, tag="ysb", bufs=3)
                nc.vector.tensor_scalar(
                    y_sb[:], p_y[:], bproj[:, m : m + 1], None, op0=TSADD
                )
                nc.gpsimd.dma_start(out=yT_d[u * N_DK + m], in_=y_sb[:])

        pending = []  # (block_id, mm_args, mm_kwargs)
        fin_prev = None  # (block_id, pair, q0, av) awaiting tail-flush + recip
        norm_prev = None  # (pair, q0, rzs) awaiting normalize
        state = dict(fin_prev=None, norm_prev=None)

        def emit_attn_block(bid, pair, s):
            q0 = 512 * s
            n_k = 4 * s + 4
            av = [
                ps.tile([65, 512], F32, tag=f"av{h}", bufs=1, name=f"av{h}")
                for h in range(2)
            ]
            for k in range(n_k):
                k0 = 128 * k
                diag_t = k - 4 * s
                lo = 128 * diag_t if diag_t >= 0 else 0
                # both heads' scores into one 2-bank psum tile; halves at
                # column offsets 0 / 512 so the off-diagonal exp covers both
                s_ps = ps.tile([128, 1024], F32, tag="st", bufs=2)
                pt = sb.tile([128, 1024], B16, tag="pt", bufs=AV_DELAY // 2 + 2)
                for h in range(2):
                    c0 = 512 * h
                    mm(
                        s_ps[:, c0 + lo : c0 + 512],
                        kz[pair][h][:, k0 : k0 + 128],
                        qT[pair][:, q0 + lo : q0 + 512],
                        start=True,
                        stop=True,
                    )
                if diag_t >= 0:
                    for h in range(2):
                        c0 = 512 * h
                        nc.scalar.activation(
                            pt[:, c0 + lo : c0 + 512],
                            s_ps[:, c0 + lo : c0 + 512],
                            mybir.ActivationFunctionType.Exp,
                            scale=SCALE,
                        )
                    # causal mask: multiply the diagonal 128x128 block of
                    # exp(S^T) by a 0/1 lower-triangle (GpSimd, sbuf-only)
                    for h in range(2):
                        c0 = 512 * h
                        nc.gpsimd.tensor_tensor(
                            pt[:, c0 + lo : c0 + lo + 128],
                            pt[:, c0 + lo : c0 + lo + 128],
                            tri[:],
                            op=mybir.AluOpType.mult,
                        )
                else:
                    nc.scalar.activation(
                        pt[:],
                        s_ps[:],
                        mybir.ActivationFunctionType.Exp,
                        scale=SCALE,
                    )
                for h in range(2):
                    hg = 2 * pair + h
                    c0 = 512 * h
                    pending.append(
                        (
                            bid,
                            (
                                av[h][0:65, lo:512],
                                vsb[k][:, 65 * hg : 65 * hg + 65],
                                pt[:, c0 + lo : c0 + 512],
                            ),
                            dict(
                                start=(k == 0),
                                stop=(k == n_k - 1),
                                skip_group_check=True,
                            ),
                        )
                    )
                    while len(pending) > AV_DELAY:
                        _, a, kw = pending.pop(0)
                        mm(*a, **kw)
                if k == 1 and state["fin_prev"] is not None:
                    # flush the previous block's tail AVs, free its av psum
                    # via recip + unnormalized copy, then run the normalize
                    # of the block before that
                    pbid = state["fin_prev"][0]
                    while pending and pending[0][0] == pbid:
                        _, a, kw = pending.pop(0)
                        mm(*a, **kw)
                    if state["norm_prev"] is not None:
                        emit_norm(*state["norm_prev"])
                        state["norm_prev"] = None
                    _, ppair, pq0, pav = state["fin_prev"]
                    state["norm_prev"] = (ppair, pq0, emit_recip(pav))
                    state["fin_prev"] = None
            state["fin_prev"] = (bid, pair, q0, av)

        bid = 0
        for s in range(NS):
            emit_qkv_chunk(s)
            if s >= 2:
                emit_proj_chunk(s - 2)
            for pair in range(2):
                emit_attn_block(bid, pair, s)
                bid += 1
        while pending:
            _, a, kw = pending.pop(0)
            mm(*a, **kw)
        if state["norm_prev"] is not None:
            emit_norm(*state["norm_prev"])
        _, ppair, pq0, pav = state["fin_prev"]
        emit_norm(ppair, pq0, emit_recip(pav))
        emit_proj_chunk(2)
        emit_proj_chunk(3)
    _split_multi_waits(nc)
    return nc


_NC_CACHE = None
LAST_RESULTS = None

_ONESR = np.ones((1, 512), dtype=NPB16)
_ZER = np.zeros((64, L), dtype=NPB16)
_I, _J = np.meshgrid(np.arange(128), np.arange(128), indexing="ij")
_TRI = (_J >= _I).astype(NPB16)  # 1.0 keep / 0.0 mask on the S^T diag block
_SEL = np.zeros((2, 128), dtype=NPB16)
_SEL[0, 0:64] = 1.0
_SEL[1, 64:128] = 1.0


def _make_in_maps(x, Wqkv, bqkv, Wproj, bproj):
    in_maps = []
    for c in range(N_CORES):
        b, g = divmod(c, 4)
        qc = slice(CD * g, CD * (g + 1))
        wq = Wqkv[:, qc]
        wk = Wqkv[:, D : 2 * D][:, qc]
        wv = Wqkv[:, 2 * D : 3 * D][:, qc]
        bq = bqkv[qc]
        bk = bqkv[D : 2 * D][qc]
        bvv = bqkv[2 * D : 3 * D][qc]
        # V columns interleaved per head: [wv_h (64 cols) | zeros col] so the
        # psum comes out in vsb layout; bv row gets [bv_h | 1.0].
        wv_i = np.zeros((D, VW), dtype=np.float32)
        bv_i = np.zeros((1, VW), dtype=np.float32)
        for h in range(HPC):
            wv_i[:, 65 * h : 65 * h + 64] = wv[:, 64 * h : 64 * h + 64]
            bv_i[0, 65 * h : 65 * h + 64] = bvv[64 * h : 64 * h + 64]
            bv_i[0, 65 * h + 64] = 1.0
        bqk_cols = np.concatenate([bq, bk]).reshape(4, 128).T  # [128, 4]
        xT = x[b].T.astype(NPB16)  # [D, L]
        xT_t = np.ascontiguousarray(
            xT.reshape(N_DK, 128, NS, 512).transpose(0, 2, 1, 3)
        ).reshape(N_DK * NS, 128, 512)
        wqkv_full = np.concatenate([wq, wk, wv_i], axis=1).astype(NPB16)
        wqkv_t = np.ascontiguousarray(wqkv_full.reshape(N_DK, 128, 2 * CD + VW))
        wproj_t = np.ascontiguousarray(
            Wproj[CD * g : CD * (g + 1), :].astype(NPB16).reshape(2, 128, D)
        )
        in_maps.append(
            {
                "xT": xT_t,
                "wqkv": wqkv_t,
                "bqk": np.ascontiguousarray(bqk_cols),
                "bv": bv_i.astype(NPB16),
                "wproj": wproj_t,
                "bproj": np.ascontiguousarray(
                    (bproj if g == 0 else np.zeros_like(bproj)).reshape(N_DK, 128).T
                ),
                "onesr": _ONESR,
                "trimask": _TRI,
                "sel": _SEL,
                "zer": _ZER,
            }
        )

    return in_maps


def kernel(x, Wqkv, bqkv, Wproj, bproj):
    global _NC_CACHE, LAST_RESULTS
    x = np.asarray(x, dtype=np.float32)
    Wqkv = np.asarray(Wqkv, dtype=np.float32)
    bqkv = np.asarray(bqkv, dtype=np.float32)
    Wproj = np.asarray(Wproj, dtype=np.float32)
    bproj = np.asarray(bproj, dtype=np.float32)

    if _NC_CACHE is None:
        _NC_CACHE = _build_program()
    nc = _NC_CACHE

    in_maps = _make_in_maps(x, Wqkv, bqkv, Wproj, bproj)
    res = run_bass_kernel_spmd(nc, in_maps, core_ids=list(range(N_CORES)))
    LAST_RESULTS = res

    out = np.empty((B, L, D), dtype=np.float32)
    for b in range(B):
        acc = res.results[4 * b]["yT"].astype(np.float32)
        for g in range(1, 4):
            acc = acc + res.results[4 * b + g]["yT"]
        # yT tiles indexed s*N_DK+m, each [128 dcols, 512 tokens]
        yT = acc.reshape(NS, N_DK, 128, 512).transpose(1, 2, 0, 3).reshape(D, L)
        out[b] = yT.T
    return out


# revision 19
# speedup vs baseline: 1.1768x; 1.1183x over previous
"""Causal self-attention (B=2, L=2048, D=1024, H=16) on 8 trn2 NeuronCores.

Sharding: core c = 4*b + g handles batch b and head group g (4 heads).
Per core: QKV projection for its heads' weight columns (tensor-parallel),
flash-style causal attention for its 4 heads, and a partial output
projection over its 256 head-dims (row-parallel).  The host sums the 4
partial projections per batch and adds bproj.

Device layout: activations kept transposed (feature-major) throughout:
  xT [D, L] (bf16, DMA'd as contiguous [128,512] tiles) -> Q^T packed per
  head-pair [128, L]; K^T stored as 4 zero-padded [128, L] bf16 tensors
  (head h occupies its 64 partition rows, the rest are zero) so score
  matmuls run at K=128 contraction; V in natural [L, 4*65] bf16 layout --
  col 64 of each head group is 1.0 so the attention row-sum Z rides along
  in the AV matmul; S^T tiles [k, q] so softmax needs no transposes; both
  heads of a pair share one [128,1024] two-bank psum tile so off-diagonal
  exp runs as a single ACT op; causal diagonal handled by a 0/1 bf16
  triangular MULTIPLY on the exp output (GpSimd, SBUF-only engine);
  softmax 1/Z computed once per block on a [2,512] tile (both heads) and
  broadcast through a selector matmul.  All matmuls bf16, fp32 psum.
Input/output DMA uses tile-contiguous DRAM layouts and is issued
round-robin across engine queues (the Sync queue alone serializes at
~0.6us per descriptor).
"""

import sys
import types

import numpy as np


def _install_ntff_shim():
    """The container's antenv stub lacks axon_hooks; recreate it so
    run_bass_kernel_spmd(trace=True) can reach the NTFF profiler."""
    if "antenv.axon_hooks" in sys.modules:
        return
    try:
        import antenv
        from trn_agent_boot.trn_boot import _ntff_profile_via_ctypes
    except Exception:
        return
    mod = types.ModuleType("antenv.axon_hooks")
    hook = _ntff_profile_via_ctypes("/opt/axon/libaxon_pjrt.so")
    mod.get_axon_ntff_profile_hook = lambda: hook
    mod.set_axon_ntff_profile_hook = lambda h: None
    sys.modules["antenv.axon_hooks"] = mod
    antenv.axon_hooks = mod


_install_ntff_shim()

import ml_dtypes  # noqa: E402

import concourse.bass as bass  # noqa: E402
import concourse.mybir as mybir  # noqa: E402
import concourse.tile as tile  # noqa: E402
from concourse.bass_utils import run_bass_kernel_spmd  # noqa: E402
from concourse.vector_clock import ScopedClock, VectorClock  # noqa: E402

B, L, D, H = 2, 2048, 1024, 16
HD = D // H  # 64
N_CORES = 8
HPC = 4  # heads per core
CD = HPC * HD  # 256 head-dims per core
VW = HPC * (HD + 1)  # 260 interleaved V columns (64 vals + ones col per head)
SCALE = HD**-0.5  # 0.125
F32 = mybir.dt.float32
B16 = mybir.dt.bfloat16
NPB16 = ml_dtypes.bfloat16

KT = L // 128  # 16 k-tiles of 128 keys
NS = L // 512  # 4 query chunks of 512
N_DK = D // 128  # 8 feature k-tiles
AV_DELAY = 6  # AV matmul issues this many (k,h)-steps behind its exp


class _TileContext(tile.TileContext):
    """Split exit-drain sem waits to 1 per drain; this walrus build's
    CTRL codegen rejects drains with 2+ sync waits."""

    def _drain_and_barrier(self, tick_clock, wait_clock):
        g = tick_clock.global_clock
        n = len(g)
        procs = [i for i in range(n) if g[i] > 0]
        for p in procs:
            vec = [g[i] if i == p else 0 for i in range(n)]
            d = self.nc.sync.drain()
            wait_clock.add_sem_waits(d.ins, ScopedClock({None: VectorClock(vec)}))
        self.nc.all_engine_barrier()
        popped = self.nc._tile_sem_poison_stack.pop()
        assert popped is self._sem_poison
        self.nc.clear_and_free_semaphores(list(self.sems.allocated().values()))
        self.nc.all_engine_barrier()


def _split_multi_waits(nc):
    """This walrus build's codegen accepts only ONE sync wait per
    instruction; hoist extra waits onto preceding same-engine NOPs."""
    for f in nc.m.functions:
        for blk in f.blocks:
            orig = list(blk.instructions)
            expanded = []
            changed = False
            for ins in orig:
                si = ins.sync_info
                if si is not None and si.on_wait is not None and len(si.on_wait) > 1:
                    changed = True
                    waits = list(si.on_wait)
                    eng = nc.engines[ins.engine]
                    for w in waits[:-1]:
                        nop = eng.nop(nofuse=True).ins
                        # eng.nop() auto-appends to the CURRENT bb; pull it
                        # out -- we re-insert it before `ins` in ins's bb.
                        nc.cur_bb.bb.instructions.remove(nop)
                        nop.sync_info = mybir.SyncInfo(on_wait=[w], on_update=[])
                        expanded.append(nop)
                    ins.sync_info = mybir.SyncInfo(
                        on_wait=[waits[-1]], on_update=list(si.on_update or [])
                    )
                expanded.append(ins)
            if changed:
                il = blk.instructions
                for ins in list(il):
                    il.remove(ins)
                for ins in expanded:
                    il.append(ins)


def _build_program():
    nc = bass.Bass()
    # tile-contiguous DRAM layouts: each DMA descriptor moves one
    # contiguous [128, C] block (large linear packets, no row striding)
    xT_d = nc.dram_tensor("xT", [N_DK * NS, 128, 512], B16, kind="ExternalInput").ap()
    wqkv_d = nc.dram_tensor(
        "wqkv", [N_DK, 128, 2 * CD + VW], B16, kind="ExternalInput"
    ).ap()
    bqk_d = nc.dram_tensor("bqk", [128, 4], F32, kind="ExternalInput").ap()
    bv_d = nc.dram_tensor("bv", [1, VW], B16, kind="ExternalInput").ap()
    wproj_d = nc.dram_tensor("wproj", [2, 128, D], B16, kind="ExternalInput").ap()
    bproj_d = nc.dram_tensor("bproj", [128, N_DK], F32, kind="ExternalInput").ap()
    onesr_d = nc.dram_tensor("onesr", [1, 512], B16, kind="ExternalInput").ap()
    tri_d = nc.dram_tensor("trimask", [128, 128], B16, kind="ExternalInput").ap()
    sel_d = nc.dram_tensor("sel", [2, 128], B16, kind="ExternalInput").ap()
    selo_d = nc.dram_tensor("selo", [128, 128], B16, kind="ExternalInput").ap()
    zer_d = nc.dram_tensor("zer", [64, L], B16, kind="ExternalInput").ap()
    yT_d = nc.dram_tensor("yT", [NS * N_DK, 128, 512], B16, kind="ExternalOutput").ap()

    mm = nc.tensor.matmul

    with _TileContext(nc) as tc, tc.tile_pool(name="sb", bufs=1) as sb, tc.tile_pool(
        name="ps", bufs=1, space="PSUM"
    ) as ps:
        # ---- constants (host-supplied; memset/affine_select of f32r
        # fail this walrus build's ISA checks) ----
        ones = sb.tile([1, 512], B16, tag="ones", bufs=1)
        nc.sync.dma_start(out=ones[:], in_=onesr_d[:])
        tri = sb.tile([128, 128], B16, tag="tri", bufs=1)
        nc.sync.dma_start(out=tri[:], in_=tri_d[:])
        sel = sb.tile([2, 128], B16, tag="sel", bufs=1)
        nc.sync.dma_start(out=sel[:], in_=sel_d[:])
        selo = sb.tile([128, 128], B16, tag="selo", bufs=1)
        nc.sync.dma_start(out=selo[:], in_=selo_d[:])
        bqk = sb.tile([128, 4], F32, tag="bqk", bufs=1)
        nc.sync.dma_start(out=bqk[:], in_=bqk_d[:])
        bv = sb.tile([1, VW], B16, tag="bv", bufs=1)
        nc.sync.dma_start(out=bv[:], in_=bv_d[:])
        bproj = sb.tile([128, N_DK], F32, tag="bproj", bufs=1)
        nc.sync.dma_start(out=bproj[:], in_=bproj_d[:])

        # ---- persistent SBUF tensors; DMA issue spread across engine
        # queues (sync alone serializes at ~0.6us/descriptor) ----
        dma_engs = [nc.sync, nc.gpsimd, nc.scalar]
        _rr = [0]

        def dma(out, in_, engs=None):
            pool = engs if engs is not None else dma_engs
            e = pool[_rr[0] % len(pool)]
            _rr[0] += 1
            e.dma_start(out=out, in_=in_)

        wqkv = [
            sb.tile([128, 2 * CD + VW], B16, tag=f"wqkv{k}", bufs=1, name=f"wqkv{k}")
            for k in range(N_DK)
        ]
        xTc = [
            [
                sb.tile([128, 512], B16, tag=f"xT{k}_{s}", bufs=1, name=f"xT{k}_{s}")
                for s in range(NS)
            ]
            for k in range(N_DK)
        ]
        # first wave: everything the s=0 QKV accumulation chains need
        for k in range(N_DK):
            dma(wqkv[k][:], wqkv_d[k])
            dma(xTc[k][0][:], xT_d[k * NS + 0])
        late_engs = [nc.sync, nc.gpsimd, nc.scalar]
        for s in range(1, NS):
            for k in range(N_DK):
                dma(xTc[k][s][:], xT_d[k * NS + s], engs=late_engs)
        wproj = []
        for kt in range(2):
            t = sb.tile([128, D], B16, tag=f"wproj{kt}", bufs=1)
            dma(t[:], wproj_d[kt], engs=late_engs)
            wproj.append(t)
        # Q^T packed per head pair (rows 0-63 = head 2p, 64-127 = head 2p+1)
        qT = [sb.tile([128, L], B16, tag=f"qT{p}", bufs=1, name=f"qT{p}") for p in range(2)]
        # K^T zero-padded per head: kz[p][h] has head 2p+h in its own 64
        # rows, zeros elsewhere -> K=128 score matmuls pick out one head.
        kz = [
            [
                sb.tile([128, L], B16, tag=f"kz{p}{h}", bufs=1, name=f"kz{p}{h}")
                for h in range(2)
            ]
            for p in range(2)
        ]
        for p in range(2):
            dma(kz[p][0][64:128, :], zer_d[:], engs=late_engs)
            dma(kz[p][1][0:64, :], zer_d[:], engs=late_engs)
        # V natural layout, 16 token tiles of [128, 4*65]; col 64 of each
        # head group = 1.0 (from interleaved W zero-cols + bias ones row)
        vsb = [sb.tile([128, VW], B16, tag=f"v{t}", bufs=1, name=f"v{t}") for t in range(KT)]
        attnT = [sb.tile([128, L], B16, tag=f"attnT{k}", bufs=1, name=f"attnT{k}") for k in range(2)]
        # Z batch tiles: 4 Z rows per group (partitions 0/32/64/96) so ONE
        # [128,512] DVE reciprocal serves two attention blocks (DVE op cost
        # is free-size * cycles regardless of partition count)
        zbs = [sb.tile([128, 512], F32, tag=f"zb{i}", bufs=1, name=f"zb{i}") for i in range(2)]
        rzbs = [sb.tile([128, 512], B16, tag=f"rzb{i}", bufs=1, name=f"rzb{i}") for i in range(2)]
        for t in zbs:
            nc.gpsimd.memset(t[:], 1.0)

        # ============== interleaved QKV / attention / proj ==============
        # Emission order per s-chunk: QKV chunk s, then proj chunk s-2,
        # then attention blocks (0,s),(1,s).  Tile deps keep it correct;
        # interleaving keeps the PE queue full (no p-state resets) and
        # spreads the ACT exp work across the whole kernel.
        TSADD = mybir.AluOpType.add

        def emit_qkv_chunk(s):
            # Q/K part: out[wcol, token] = wqkv[:, m-tile].T @ xT
            for m in range(4):
                p_qk = ps.tile([128, 512], F32, tag="mm", bufs=2)
                for k in range(N_DK):
                    mm(
                        p_qk[:],
                        wqkv[k][:, 128 * m : 128 * (m + 1)],
                        xTc[k][s][:],
                        start=(k == 0),
                        stop=(k == N_DK - 1),
                    )
                # copy to SBUF (bf16) with per-partition (wcol) bias add
                cs = slice(512 * s, 512 * (s + 1))
                if m < 2:
                    nc.vector.tensor_scalar(
                        qT[m][:, cs], p_qk[:], bqk[:, m : m + 1], None, op0=TSADD
                    )
                else:
                    p = m - 2
                    nc.vector.tensor_scalar(
                        kz[p][0][0:64, cs], p_qk[0:64, :],
                        bqk[0:64, m : m + 1], None, op0=TSADD,
                    )
                    nc.vector.tensor_scalar(
                        kz[p][1][64:128, cs], p_qk[64:128, :],
                        bqk[64:128, m : m + 1], None, op0=TSADD,
                    )
            # V part: out[token, vcol] = xT[:, tt].T @ wv_interleaved
            for j in range(4):
                t = 4 * s + j
                p_v = ps.tile([128, VW], F32, tag="mm", bufs=2)
                for k in range(N_DK):
                    mm(
                        p_v[:],
                        xTc[k][s][:, 128 * j : 128 * (j + 1)],
                        wqkv[k][:, 2 * CD : 2 * CD + VW],
                        start=(k == 0),
                        stop=False,
                    )
                mm(p_v[:], ones[0:1, 0:128], bv[:], start=False, stop=True)
                nc.scalar.copy(vsb[t][:], p_v[:])

        def emit_flush(av, pair, s):
            # block end: pull Z rows into the group tile (partition
            # 64*pair+32*h) and the unnormalized AV rows out of psum
            zb = zbs[s % 2]
            uns = []
            for h in range(2):
                r = 64 * pair + 32 * h
                nc.vector.tensor_copy(zb[r : r + 1, :], av[h][64:65, :])
                un = sb.tile([64, 512], F32, tag="un", bufs=6, name="un")
                nc.vector.tensor_copy(un[:], av[h][0:64, :])
                uns.append(un)
            return uns

        def emit_group_recip(s):
            # one reciprocal for all 4 Z rows of query-chunk s
            with nc.allow_low_precision(reason="1/Z in bf16"):
                nc.vector.reciprocal(rzbs[s % 2][:], zbs[s % 2][:])

        def emit_norm(pair, q0, s, uns):
            rzb = rzbs[s % 2]
            for h in range(2):
                b0 = 64 * pair
                # broadcast 1/Z_h (row 64*pair+32*h of rzb) via a K=64
                # selector matmul; psum held only for the short DVE multiply
                bc_ps = ps.tile([64, 512], F32, tag="mm", bufs=2, name="bc_ps")
                mm(
                    bc_ps[:],
                    selo[b0 : b0 + 64, 64 * h : 64 * h + 64],
                    rzb[b0 : b0 + 64, :],
                    start=True,
                    stop=True,
                )
                if h == 0:
                    nc.vector.tensor_tensor(
                        attnT[pair][0:64, q0 : q0 + 512],
                        uns[h][:],
                        bc_ps[:],
                        op=mybir.AluOpType.mult,
                    )
                else:
                    tmp = sb.tile([64, 512], B16, tag="ntmp", bufs=2, name="tmp")
                    nc.vector.tensor_tensor(
                        tmp[:], uns[h][:], bc_ps[:], op=mybir.AluOpType.mult
                    )
                    nc.gpsimd.dma_start(
                        out=attnT[pair][64:128, q0 : q0 + 512], in_=tmp[:]
                    )

        def emit_proj_chunk(u):
            for m in range(N_DK):
                p_y = ps.tile([128, 512], F32, tag="mm", bufs=2)
                for kt in range(2):
                    mm(
                        p_y[:],
                        wproj[kt][:, 128 * m : 128 * (m + 1)],
                        attnT[kt][:, 512 * u : 512 * (u + 1)],
                        start=(kt == 0),
                        stop=(kt == 1),
                    )
                y_sb = sb.tile([128, 512], B16, tag="ysb", bufs=3)
                nc.vector.tensor_scalar(
                    y_sb[:], p_y[:], bproj[:, m : m + 1], None, op0=TSADD
                )
                nc.gpsimd.dma_start(out=yT_d[u * N_DK + m], in_=y_sb[:])

        pending = []  # (block_id, mm_args, mm_kwargs)
        state = dict(fin_prev=None, uns={})

        def emit_attn_block(bid, pair, s):
            q0 = 512 * s
            n_k = 4 * s + 4
            av = [
                ps.tile([65, 512], F32, tag=f"av{h}", bufs=1, name=f"av{h}")
                for h in range(2)
            ]
            for k in range(n_k):
                k0 = 128 * k
                diag_t = k - 4 * s
                lo = 128 * diag_t if diag_t >= 0 else 0
                # both heads' scores into one 2-bank psum tile; halves at
                # column offsets 0 / 512 so the off-diagonal exp covers both
                s_ps = ps.tile([128, 1024], F32, tag="st", bufs=2)
                pt = sb.tile([128, 1024], B16, tag="pt", bufs=AV_DELAY // 2 + 2)
                for h in range(2):
                    c0 = 512 * h
                    mm(
                        s_ps[:, c0 + lo : c0 + 512],
                        kz[pair][h][:, k0 : k0 + 128],
                        qT[pair][:, q0 + lo : q0 + 512],
                        start=True,
                        stop=True,
                    )
                if diag_t >= 0:
                    for h in range(2):
                        c0 = 512 * h
                        nc.scalar.activation(
                            pt[:, c0 + lo : c0 + 512],
                            s_ps[:, c0 + lo : c0 + 512],
                            mybir.ActivationFunctionType.Exp,
                            scale=SCALE,
                        )
                    # causal mask: multiply the diagonal 128x128 block of
                    # exp(S^T) by a 0/1 lower-triangle (GpSimd, sbuf-only)
                    for h in range(2):
                        c0 = 512 * h
                        nc.gpsimd.tensor_tensor(
                            pt[:, c0 + lo : c0 + lo + 128],
                            pt[:, c0 + lo : c0 + lo + 128],
                            tri[:],
                            op=mybir.AluOpType.mult,
                        )
                else:
                    nc.scalar.activation(
                        pt[:],
                        s_ps[:],
                        mybir.ActivationFunctionType.Exp,
                        scale=SCALE,
                    )
                for h in range(2):
                    hg = 2 * pair + h
                    c0 = 512 * h
                    pending.append(
                        (
                            bid,
                            (
                                av[h][0:65, lo:512],
                                vsb[k][:, 65 * hg : 65 * hg + 65],
                                pt[:, c0 + lo : c0 + 512],
                            ),
                            dict(
                                start=(k == 0),
                                stop=(k == n_k - 1),
                                skip_group_check=True,
                            ),
                        )
                    )
                    while len(pending) > AV_DELAY:
                        _, a, kw = pending.pop(0)
                        mm(*a, **kw)
                if k == 1 and state["fin_prev"] is not None:
                    # flush the previous block's tail AVs and free its av
                    # psum; once both blocks of a query chunk are flushed,
                    # run the shared reciprocal + both normalizes
                    pbid = state["fin_prev"][0]
                    while pending and pending[0][0] == pbid:
                        _, a, kw = pending.pop(0)
                        mm(*a, **kw)
                    _, ppair, ps_, pav = state["fin_prev"]
                    uns = emit_flush(pav, ppair, ps_)
                    state["uns"][(ppair, ps_)] = uns
                    if ppair == 1:
                        emit_group_recip(ps_)
                        for npair in range(2):
                            emit_norm(
                                npair, 512 * ps_, ps_,
                                state["uns"].pop((npair, ps_)),
                            )
                    state["fin_prev"] = None
            state["fin_prev"] = (bid, pair, s, av)

        bid = 0
        for s in range(NS):
            emit_qkv_chunk(s)
            if s >= 2:
                emit_proj_chunk(s - 2)
            for pair in range(2):
                emit_attn_block(bid, pair, s)
                bid += 1
        while pending:
            _, a, kw = pending.pop(0)
            mm(*a, **kw)
        _, ppair, ps_, pav = state["fin_prev"]
        uns = emit_flush(pav, ppair, ps_)
        state["uns"][(ppair, ps_)] = uns
        emit_group_recip(ps_)
        for npair in range(2):
            emit_norm(npair, 512 * ps_, ps_, state["uns"].pop((npair, ps_)))
        emit_proj_chunk(2)
        emit_proj_chunk(3)
    _split_multi_waits(nc)
    return nc


_NC_CACHE = None
LAST_RESULTS = None

_ONESR = np.ones((1, 512), dtype=NPB16)
_ZER = np.zeros((64, L), dtype=NPB16)
_I, _J = np.meshgrid(np.arange(128), np.arange(128), indexing="ij")
_TRI = (_J >= _I).astype(NPB16)  # 1.0 keep / 0.0 mask on the S^T diag block
_SELO = np.zeros((128, 128), dtype=NPB16)
_SELO[0, 0:64] = 1.0
_SELO[64, 0:64] = 1.0
_SELO[32, 64:128] = 1.0
_SELO[96, 64:128] = 1.0
_SEL = np.zeros((2, 128), dtype=NPB16)
_SEL[0, 0:64] = 1.0
_SEL[1, 64:128] = 1.0


def _make_in_maps(x, Wqkv, bqkv, Wproj, bproj):
    in_maps = []
    for c in range(N_CORES):
        b, g = divmod(c, 4)
        qc = slice(CD * g, CD * (g + 1))
        wq = Wqkv[:, qc]
        wk = Wqkv[:, D : 2 * D][:, qc]
        wv = Wqkv[:, 2 * D : 3 * D][:, qc]
        bq = bqkv[qc]
        bk = bqkv[D : 2 * D][qc]
        bvv = bqkv[2 * D : 3 * D][qc]
        # V columns interleaved per head: [wv_h (64 cols) | zeros col] so the
        # psum comes out in vsb layout; bv row gets [bv_h | 1.0].
        wv_i = np.zeros((D, VW), dtype=np.float32)
        bv_i = np.zeros((1, VW), dtype=np.float32)
        for h in range(HPC):
            wv_i[:, 65 * h : 65 * h + 64] = wv[:, 64 * h : 64 * h + 64]
            bv_i[0, 65 * h : 65 * h + 64] = bvv[64 * h : 64 * h + 64]
            bv_i[0, 65 * h + 64] = 1.0
        bqk_cols = np.concatenate([bq, bk]).reshape(4, 128).T  # [128, 4]
        xT = x[b].T.astype(NPB16)  # [D, L]
        xT_t = np.ascontiguousarray(
            xT.reshape(N_DK, 128, NS, 512).transpose(0, 2, 1, 3)
        ).reshape(N_DK * NS, 128, 512)
        wqkv_full = np.concatenate([wq, wk, wv_i], axis=1).astype(NPB16)
        wqkv_t = np.ascontiguousarray(wqkv_full.reshape(N_DK, 128, 2 * CD + VW))
        wproj_t = np.ascontiguousarray(
            Wproj[CD * g : CD * (g + 1), :].astype(NPB16).reshape(2, 128, D)
        )
        in_maps.append(
            {
                "xT": xT_t,
                "wqkv": wqkv_t,
                "bqk": np.ascontiguousarray(bqk_cols),
                "bv": bv_i.astype(NPB16),
                "wproj": wproj_t,
                "bproj": np.ascontiguousarray(
                    (bproj if g == 0 else np.zeros_like(bproj)).reshape(N_DK, 128).T
                ),
                "onesr": _ONESR,
                "trimask": _TRI,
                "sel": _SEL,
                "selo": _SELO,
                "zer": _ZER,
            }
        )

    return in_maps


def kernel(x, Wqkv, bqkv, Wproj, bproj):
    global _NC_CACHE, LAST_RESULTS
    x = np.asarray(x, dtype=np.float32)
    Wqkv = np.asarray(Wqkv, dtype=np.float32)
    bqkv = np.asarray(bqkv, dtype=np.float32)
    Wproj = np.asarray(Wproj, dtype=np.float32)
    bproj = np.asarray(bproj, dtype=np.float32)

    if _NC_CACHE is None:
        _NC_CACHE = _build_program()
    nc = _NC_CACHE

    in_maps = _make_in_maps(x, Wqkv, bqkv, Wproj, bproj)
    res = run_bass_kernel_spmd(nc, in_maps, core_ids=list(range(N_CORES)))
    LAST_RESULTS = res

    out = np.empty((B, L, D), dtype=np.float32)
    for b in range(B):
        acc = res.results[4 * b]["yT"].astype(np.float32)
        for g in range(1, 4):
            acc = acc + res.results[4 * b + g]["yT"]
        # yT tiles indexed s*N_DK+m, each [128 dcols, 512 tokens]
        yT = acc.reshape(NS, N_DK, 128, 512).transpose(1, 2, 0, 3).reshape(D, L)
        out[b] = yT.T
    return out


# revision 20
# speedup vs baseline: 1.2047x; 1.0237x over previous
"""Causal self-attention (B=2, L=2048, D=1024, H=16) on 8 trn2 NeuronCores.

Sharding: core c = 4*b + g handles batch b and head group g (4 heads).
Per core: QKV projection for its heads' weight columns (tensor-parallel),
flash-style causal attention for its 4 heads, and a partial output
projection over its 256 head-dims (row-parallel).  The host sums the 4
partial projections per batch and adds bproj.

Device layout: activations kept transposed (feature-major) throughout:
  xT [D, L] (bf16, DMA'd as contiguous [128,512] tiles) -> Q^T packed per
  head-pair [128, L]; K^T stored as 4 zero-padded [128, L] bf16 tensors
  (head h occupies its 64 partition rows, the rest are zero) so score
  matmuls run at K=128 contraction; V in natural [L, 4*65] bf16 layout --
  col 64 of each head group is 1.0 so the attention row-sum Z rides along
  in the AV matmul; S^T tiles [k, q] so softmax needs no transposes; both
  heads of a pair share one [128,1024] two-bank psum tile so off-diagonal
  exp runs as a single ACT op; causal diagonal handled by a 0/1 bf16
  triangular MULTIPLY on the exp output (GpSimd, SBUF-only engine);
  softmax 1/Z computed once per block on a [2,512] tile (both heads) and
  broadcast through a selector matmul.  All matmuls bf16, fp32 psum.
Input/output DMA uses tile-contiguous DRAM layouts and is issued
round-robin across engine queues (the Sync queue alone serializes at
~0.6us per descriptor).
"""

import sys
import types

import numpy as np


def _install_ntff_shim():
    """The container's antenv stub lacks axon_hooks; recreate it so
    run_bass_kernel_spmd(trace=True) can reach the NTFF profiler."""
    if "antenv.axon_hooks" in sys.modules:
        return
    try:
        import antenv
        from trn_agent_boot.trn_boot import _ntff_profile_via_ctypes
    except Exception:
        return
    mod = types.ModuleType("antenv.axon_hooks")
    hook = _ntff_profile_via_ctypes("/opt/axon/libaxon_pjrt.so")
    mod.get_axon_ntff_profile_hook = lambda: hook
    mod.set_axon_ntff_profile_hook = lambda h: None
    sys.modules["antenv.axon_hooks"] = mod
    antenv.axon_hooks = mod


_install_ntff_shim()

import ml_dtypes  # noqa: E402

import concourse.bass as bass  # noqa: E402
import concourse.mybir as mybir  # noqa: E402
import concourse.tile as tile  # noqa: E402
from concourse.bass_utils import run_bass_kernel_spmd  # noqa: E402
from concourse.vector_clock import ScopedClock, VectorClock  # noqa: E402

B, L, D, H = 2, 2048, 1024, 16
HD = D // H  # 64
N_CORES = 8
HPC = 4  # heads per core
CD = HPC * HD  # 256 head-dims per core
VW = HPC * (HD + 1)  # 260 interleaved V columns (64 vals + ones col per head)
SCALE = HD**-0.5  # 0.125
F32 = mybir.dt.float32
B16 = mybir.dt.bfloat16
NPB16 = ml_dtypes.bfloat16

KT = L // 128  # 16 k-tiles of 128 keys
NS = L // 512  # 4 query chunks of 512
N_DK = D // 128  # 8 feature k-tiles
AV_DELAY = 8  # AV matmul issues this many (k,h)-steps behind its exp


class _TileContext(tile.TileContext):
    """Split exit-drain sem waits to 1 per drain; this walrus build's
    CTRL codegen rejects drains with 2+ sync waits."""

    def _drain_and_barrier(self, tick_clock, wait_clock):
        g = tick_clock.global_clock
        n = len(g)
        procs = [i for i in range(n) if g[i] > 0]
        for p in procs:
            vec = [g[i] if i == p else 0 for i in range(n)]
            d = self.nc.sync.drain()
            wait_clock.add_sem_waits(d.ins, ScopedClock({None: VectorClock(vec)}))
        self.nc.all_engine_barrier()
        popped = self.nc._tile_sem_poison_stack.pop()
        assert popped is self._sem_poison
        self.nc.clear_and_free_semaphores(list(self.sems.allocated().values()))
        self.nc.all_engine_barrier()


def _split_multi_waits(nc):
    """This walrus build's codegen accepts only ONE sync wait per
    instruction; hoist extra waits onto preceding same-engine NOPs."""
    for f in nc.m.functions:
        for blk in f.blocks:
            orig = list(blk.instructions)
            expanded = []
            changed = False
            for ins in orig:
                si = ins.sync_info
                if si is not None and si.on_wait is not None and len(si.on_wait) > 1:
                    changed = True
                    waits = list(si.on_wait)
                    eng = nc.engines[ins.engine]
                    for w in waits[:-1]:
                        nop = eng.nop(nofuse=True).ins
                        # eng.nop() auto-appends to the CURRENT bb; pull it
                        # out -- we re-insert it before `ins` in ins's bb.
                        nc.cur_bb.bb.instructions.remove(nop)
                        nop.sync_info = mybir.SyncInfo(on_wait=[w], on_update=[])
                        expanded.append(nop)
                    ins.sync_info = mybir.SyncInfo(
                        on_wait=[waits[-1]], on_update=list(si.on_update or [])
                    )
                expanded.append(ins)
            if changed:
                il = blk.instructions
                for ins in list(il):
                    il.remove(ins)
                for ins in expanded:
                    il.append(ins)


def _build_program():
    nc = bass.Bass()
    # tile-contiguous DRAM layouts: each DMA descriptor moves one
    # contiguous [128, C] block (large linear packets, no row striding)
    xT_d = nc.dram_tensor("xT", [N_DK * NS, 128, 512], B16, kind="ExternalInput").ap()
    wqkv_d = nc.dram_tensor(
        "wqkv", [N_DK, 128, 2 * CD + VW], B16, kind="ExternalInput"
    ).ap()
    bqk_d = nc.dram_tensor("bqk", [128, 4], F32, kind="ExternalInput").ap()
    bv_d = nc.dram_tensor("bv", [1, VW], B16, kind="ExternalInput").ap()
    wproj_d = nc.dram_tensor("wproj", [2, 128, D], B16, kind="ExternalInput").ap()
    bproj_d = nc.dram_tensor("bproj", [128, N_DK], F32, kind="ExternalInput").ap()
    onesr_d = nc.dram_tensor("onesr", [1, 512], B16, kind="ExternalInput").ap()
    tri_d = nc.dram_tensor("trimask", [128, 128], B16, kind="ExternalInput").ap()
    sel_d = nc.dram_tensor("sel", [2, 128], B16, kind="ExternalInput").ap()
    selo_d = nc.dram_tensor("selo", [128, 128], B16, kind="ExternalInput").ap()
    zer_d = nc.dram_tensor("zer", [64, L], B16, kind="ExternalInput").ap()
    yT_d = nc.dram_tensor("yT", [NS * N_DK, 128, 512], B16, kind="ExternalOutput").ap()

    mm = nc.tensor.matmul

    with _TileContext(nc) as tc, tc.tile_pool(name="sb", bufs=1) as sb, tc.tile_pool(
        name="ps", bufs=1, space="PSUM"
    ) as ps:
        # ---- constants (host-supplied; memset/affine_select of f32r
        # fail this walrus build's ISA checks) ----
        ones = sb.tile([1, 512], B16, tag="ones", bufs=1)
        nc.sync.dma_start(out=ones[:], in_=onesr_d[:])
        tri = sb.tile([128, 128], B16, tag="tri", bufs=1)
        nc.sync.dma_start(out=tri[:], in_=tri_d[:])
        sel = sb.tile([2, 128], B16, tag="sel", bufs=1)
        nc.sync.dma_start(out=sel[:], in_=sel_d[:])
        selo = sb.tile([128, 128], B16, tag="selo", bufs=1)
        nc.sync.dma_start(out=selo[:], in_=selo_d[:])
        bqk = sb.tile([128, 4], F32, tag="bqk", bufs=1)
        nc.sync.dma_start(out=bqk[:], in_=bqk_d[:])
        bv = sb.tile([1, VW], B16, tag="bv", bufs=1)
        nc.sync.dma_start(out=bv[:], in_=bv_d[:])
        bproj = sb.tile([128, N_DK], F32, tag="bproj", bufs=1)
        nc.sync.dma_start(out=bproj[:], in_=bproj_d[:])

        # ---- persistent SBUF tensors; DMA issue spread across engine
        # queues (sync alone serializes at ~0.6us/descriptor) ----
        dma_engs = [nc.sync, nc.gpsimd, nc.scalar]
        _rr = [0]

        def dma(out, in_, engs=None):
            pool = engs if engs is not None else dma_engs
            e = pool[_rr[0] % len(pool)]
            _rr[0] += 1
            e.dma_start(out=out, in_=in_)

        wqkv = [
            sb.tile([128, 2 * CD + VW], B16, tag=f"wqkv{k}", bufs=1, name=f"wqkv{k}")
            for k in range(N_DK)
        ]
        xTc = [
            [
                sb.tile([128, 512], B16, tag=f"xT{k}_{s}", bufs=1, name=f"xT{k}_{s}")
                for s in range(NS)
            ]
            for k in range(N_DK)
        ]
        # first wave: everything the s=0 QKV accumulation chains need,
        # split into half-tile descriptors to spread across more hw queues
        for k in range(N_DK):
            dma(wqkv[k][0:64, :], wqkv_d[k][0:64, :])
            dma(wqkv[k][64:128, :], wqkv_d[k][64:128, :])
            dma(xTc[k][0][0:64, :], xT_d[k * NS + 0][0:64, :])
            dma(xTc[k][0][64:128, :], xT_d[k * NS + 0][64:128, :])
        late_engs = [nc.sync, nc.gpsimd, nc.scalar]
        for s in range(1, NS):
            for k in range(N_DK):
                dma(xTc[k][s][:], xT_d[k * NS + s], engs=late_engs)
        wproj = []
        for kt in range(2):
            t = sb.tile([128, D], B16, tag=f"wproj{kt}", bufs=1)
            dma(t[:], wproj_d[kt], engs=late_engs)
            wproj.append(t)
        # Q^T packed per head pair (rows 0-63 = head 2p, 64-127 = head 2p+1)
        qT = [sb.tile([128, L], B16, tag=f"qT{p}", bufs=1, name=f"qT{p}") for p in range(2)]
        # K^T zero-padded per head: kz[p][h] has head 2p+h in its own 64
        # rows, zeros elsewhere -> K=128 score matmuls pick out one head.
        kz = [
            [
                sb.tile([128, L], B16, tag=f"kz{p}{h}", bufs=1, name=f"kz{p}{h}")
                for h in range(2)
            ]
            for p in range(2)
        ]
        for p in range(2):
            dma(kz[p][0][64:128, :], zer_d[:], engs=late_engs)
            dma(kz[p][1][0:64, :], zer_d[:], engs=late_engs)
        # V natural layout, 16 token tiles of [128, 4*65]; col 64 of each
        # head group = 1.0 (from interleaved W zero-cols + bias ones row)
        vsb = [sb.tile([128, VW], B16, tag=f"v{t}", bufs=1, name=f"v{t}") for t in range(KT)]
        attnT = [sb.tile([128, L], B16, tag=f"attnT{k}", bufs=1, name=f"attnT{k}") for k in range(2)]
        # Z batch tiles: 4 Z rows per group (partitions 0/32/64/96) so ONE
        # [128,512] DVE reciprocal serves two attention blocks (DVE op cost
        # is free-size * cycles regardless of partition count)
        zbs = [sb.tile([128, 512], F32, tag=f"zb{i}", bufs=1, name=f"zb{i}") for i in range(2)]
        rzbs = [sb.tile([128, 512], B16, tag=f"rzb{i}", bufs=1, name=f"rzb{i}") for i in range(2)]
        for t in zbs:
            nc.gpsimd.memset(t[:], 1.0)

        # ============== interleaved QKV / attention / proj ==============
        # Emission order per s-chunk: QKV chunk s, then proj chunk s-2,
        # then attention blocks (0,s),(1,s).  Tile deps keep it correct;
        # interleaving keeps the PE queue full (no p-state resets) and
        # spreads the ACT exp work across the whole kernel.
        TSADD = mybir.AluOpType.add

        def emit_qkv_chunk(s):
            # Q/K part: out[wcol, token] = wqkv[:, m-tile].T @ xT
            for m in range(4):
                p_qk = ps.tile([128, 512], F32, tag="mm", bufs=2)
                for k in range(N_DK):
                    mm(
                        p_qk[:],
                        wqkv[k][:, 128 * m : 128 * (m + 1)],
                        xTc[k][s][:],
                        start=(k == 0),
                        stop=(k == N_DK - 1),
                    )
                # copy to SBUF (bf16) with per-partition (wcol) bias add
                cs = slice(512 * s, 512 * (s + 1))
                if m < 2:
                    nc.vector.tensor_scalar(
                        qT[m][:, cs], p_qk[:], bqk[:, m : m + 1], None, op0=TSADD
                    )
                else:
                    p = m - 2
                    nc.vector.tensor_scalar(
                        kz[p][0][0:64, cs], p_qk[0:64, :],
                        bqk[0:64, m : m + 1], None, op0=TSADD,
                    )
                    nc.vector.tensor_scalar(
                        kz[p][1][64:128, cs], p_qk[64:128, :],
                        bqk[64:128, m : m + 1], None, op0=TSADD,
                    )
            # V part: out[token, vcol] = xT[:, tt].T @ wv_interleaved
            for j in range(4):
                t = 4 * s + j
                p_v = ps.tile([128, VW], F32, tag="mm", bufs=2)
                for k in range(N_DK):
                    mm(
                        p_v[:],
                        xTc[k][s][:, 128 * j : 128 * (j + 1)],
                        wqkv[k][:, 2 * CD : 2 * CD + VW],
                        start=(k == 0),
                        stop=False,
                    )
                mm(p_v[:], ones[0:1, 0:128], bv[:], start=False, stop=True)
                nc.scalar.copy(vsb[t][:], p_v[:])

        def emit_flush(av, pair, s):
            # block end: pull Z rows into the group tile (partition
            # 64*pair+32*h) and the unnormalized AV rows out of psum
            zb = zbs[s % 2]
            uns = []
            for h in range(2):
                r = 64 * pair + 32 * h
                nc.vector.tensor_copy(zb[r : r + 1, :], av[h][64:65, :])
                un = sb.tile([64, 512], F32, tag="un", bufs=6, name="un")
                nc.vector.tensor_copy(un[:], av[h][0:64, :])
                uns.append(un)
            return uns

        def emit_group_recip(s):
            # one reciprocal for all 4 Z rows of query-chunk s
            with nc.allow_low_precision(reason="1/Z in bf16"):
                nc.vector.reciprocal(rzbs[s % 2][:], zbs[s % 2][:])

        def emit_norm(pair, q0, s, uns):
            rzb = rzbs[s % 2]
            for h in range(2):
                b0 = 64 * pair
                # broadcast 1/Z_h (row 64*pair+32*h of rzb) via a K=64
                # selector matmul; psum held only for the short DVE multiply
                bc_ps = ps.tile([64, 512], F32, tag="mm", bufs=2, name="bc_ps")
                mm(
                    bc_ps[:],
                    selo[b0 : b0 + 64, 64 * h : 64 * h + 64],
                    rzb[b0 : b0 + 64, :],
                    start=True,
                    stop=True,
                )
                if h == 0:
                    nc.vector.tensor_tensor(
                        attnT[pair][0:64, q0 : q0 + 512],
                        uns[h][:],
                        bc_ps[:],
                        op=mybir.AluOpType.mult,
                    )
                else:
                    tmp = sb.tile([64, 512], B16, tag="ntmp", bufs=2, name="tmp")
                    nc.vector.tensor_tensor(
                        tmp[:], uns[h][:], bc_ps[:], op=mybir.AluOpType.mult
                    )
                    nc.gpsimd.dma_start(
                        out=attnT[pair][64:128, q0 : q0 + 512], in_=tmp[:]
                    )

        def emit_proj_chunk(u):
            for m in range(N_DK):
                p_y = ps.tile([128, 512], F32, tag="mm", bufs=2)
                for kt in range(2):
                    mm(
                        p_y[:],
                        wproj[kt][:, 128 * m : 128 * (m + 1)],
                        attnT[kt][:, 512 * u : 512 * (u + 1)],
                        start=(kt == 0),
                        stop=(kt == 1),
                    )
                y_sb = sb.tile([128, 512], B16, tag="ysb", bufs=6)
                nc.vector.tensor_scalar(
                    y_sb[:], p_y[:], bproj[:, m : m + 1], None, op0=TSADD
                )
                nc.gpsimd.dma_start(out=yT_d[u * N_DK + m], in_=y_sb[:])

        pending = []  # (block_id, mm_args, mm_kwargs)
        state = dict(fin_prev=None, uns={})

        def emit_attn_block(bid, pair, s):
            q0 = 512 * s
            n_k = 4 * s + 4
            av = [
                ps.tile([65, 512], F32, tag=f"av{h}", bufs=1, name=f"av{h}")
                for h in range(2)
            ]
            for k in range(n_k):
                k0 = 128 * k
                diag_t = k - 4 * s
                lo = 128 * diag_t if diag_t >= 0 else 0
                # both heads' scores into one 2-bank psum tile; halves at
                # column offsets 0 / 512 so the off-diagonal exp covers both
                s_ps = ps.tile([128, 1024], F32, tag="st", bufs=2)
                pt = sb.tile([128, 1024], B16, tag="pt", bufs=AV_DELAY // 2 + 2)
                for h in range(2):
                    c0 = 512 * h
                    mm(
                        s_ps[:, c0 + lo : c0 + 512],
                        kz[pair][h][:, k0 : k0 + 128],
                        qT[pair][:, q0 + lo : q0 + 512],
                        start=True,
                        stop=True,
                    )
                if diag_t >= 0:
                    for h in range(2):
                        c0 = 512 * h
                        nc.scalar.activation(
                            pt[:, c0 + lo : c0 + 512],
                            s_ps[:, c0 + lo : c0 + 512],
                            mybir.ActivationFunctionType.Exp,
                            scale=SCALE,
                        )
                    # causal mask: multiply the diagonal 128x128 block of
                    # exp(S^T) by a 0/1 lower-triangle (GpSimd, sbuf-only)
                    for h in range(2):
                        c0 = 512 * h
                        nc.gpsimd.tensor_tensor(
                            pt[:, c0 + lo : c0 + lo + 128],
                            pt[:, c0 + lo : c0 + lo + 128],
                            tri[:],
                            op=mybir.AluOpType.mult,
                        )
                else:
                    nc.scalar.activation(
                        pt[:],
                        s_ps[:],
                        mybir.ActivationFunctionType.Exp,
                        scale=SCALE,
                    )
                for h in range(2):
                    hg = 2 * pair + h
                    c0 = 512 * h
                    pending.append(
                        (
                            bid,
                            (
                                av[h][0:65, lo:512],
                                vsb[k][:, 65 * hg : 65 * hg + 65],
                                pt[:, c0 + lo : c0 + 512],
                            ),
                            dict(
                                start=(k == 0),
                                stop=(k == n_k - 1),
                                skip_group_check=True,
                            ),
                        )
                    )
                    while len(pending) > AV_DELAY:
                        _, a, kw = pending.pop(0)
                        mm(*a, **kw)
                if k == 1 and state["fin_prev"] is not None:
                    # flush the previous block's tail AVs and free its av
                    # psum; once both blocks of a query chunk are flushed,
                    # run the shared reciprocal + both normalizes
                    pbid = state["fin_prev"][0]
                    while pending and pending[0][0] == pbid:
                        _, a, kw = pending.pop(0)
                        mm(*a, **kw)
                    _, ppair, ps_, pav = state["fin_prev"]
                    uns = emit_flush(pav, ppair, ps_)
                    state["uns"][(ppair, ps_)] = uns
                    if ppair == 1:
                        emit_group_recip(ps_)
                        for npair in range(2):
                            emit_norm(
                                npair, 512 * ps_, ps_,
                                state["uns"].pop((npair, ps_)),
                            )
                    state["fin_prev"] = None
            state["fin_prev"] = (bid, pair, s, av)

        bid = 0
        for s in range(NS):
            emit_qkv_chunk(s)
            if s >= 2:
                emit_proj_chunk(s - 2)
            for pair in range(2):
                emit_attn_block(bid, pair, s)
                bid += 1
        while pending:
            _, a, kw = pending.pop(0)
            mm(*a, **kw)
        _, ppair, ps_, pav = state["fin_prev"]
        uns = emit_flush(pav, ppair, ps_)
        state["uns"][(ppair, ps_)] = uns
        emit_group_recip(ps_)
        for npair in range(2):
            emit_norm(npair, 512 * ps_, ps_, state["uns"].pop((npair, ps_)))
        emit_proj_chunk(2)
        emit_proj_chunk(3)
    _split_multi_waits(nc)
    return nc


_NC_CACHE = None
LAST_RESULTS = None

_ONESR = np.ones((1, 512), dtype=NPB16)
_ZER = np.zeros((64, L), dtype=NPB16)
_I, _J = np.meshgrid(np.arange(128), np.arange(128), indexing="ij")
_TRI = (_J >= _I).astype(NPB16)  # 1.0 keep / 0.0 mask on the S^T diag block
_SELO = np.zeros((128, 128), dtype=NPB16)
_SELO[0, 0:64] = 1.0
_SELO[64, 0:64] = 1.0
_SELO[32, 64:128] = 1.0
_SELO[96, 64:128] = 1.0
_SEL = np.zeros((2, 128), dtype=NPB16)
_SEL[0, 0:64] = 1.0
_SEL[1, 64:128] = 1.0


def _make_in_maps(x, Wqkv, bqkv, Wproj, bproj):
    in_maps = []
    for c in range(N_CORES):
        b, g = divmod(c, 4)
        qc = slice(CD * g, CD * (g + 1))
        wq = Wqkv[:, qc]
        wk = Wqkv[:, D : 2 * D][:, qc]
        wv = Wqkv[:, 2 * D : 3 * D][:, qc]
        bq = bqkv[qc]
        bk = bqkv[D : 2 * D][qc]
        bvv = bqkv[2 * D : 3 * D][qc]
        # V columns interleaved per head: [wv_h (64 cols) | zeros col] so the
        # psum comes out in vsb layout; bv row gets [bv_h | 1.0].
        wv_i = np.zeros((D, VW), dtype=np.float32)
        bv_i = np.zeros((1, VW), dtype=np.float32)
        for h in range(HPC):
            wv_i[:, 65 * h : 65 * h + 64] = wv[:, 64 * h : 64 * h + 64]
            bv_i[0, 65 * h : 65 * h + 64] = bvv[64 * h : 64 * h + 64]
            bv_i[0, 65 * h + 64] = 1.0
        bqk_cols = np.concatenate([bq, bk]).reshape(4, 128).T  # [128, 4]
        xT = x[b].T.astype(NPB16)  # [D, L]
        xT_t = np.ascontiguousarray(
            xT.reshape(N_DK, 128, NS, 512).transpose(0, 2, 1, 3)
        ).reshape(N_DK * NS, 128, 512)
        wqkv_full = np.concatenate([wq, wk, wv_i], axis=1).astype(NPB16)
        wqkv_t = np.ascontiguousarray(wqkv_full.reshape(N_DK, 128, 2 * CD + VW))
        wproj_t = np.ascontiguousarray(
            Wproj[CD * g : CD * (g + 1), :].astype(NPB16).reshape(2, 128, D)
        )
        in_maps.append(
            {
                "xT": xT_t,
                "wqkv": wqkv_t,
                "bqk": np.ascontiguousarray(bqk_cols),
                "bv": bv_i.astype(NPB16),
                "wproj": wproj_t,
                "bproj": np.ascontiguousarray(
                    (bproj if g == 0 else np.zeros_like(bproj)).reshape(N_DK, 128).T
                ),
                "onesr": _ONESR,
                "trimask": _TRI,
                "sel": _SEL,
                "selo": _SELO,
                "zer": _ZER,
            }
        )

    return in_maps


def kernel(x, Wqkv, bqkv, Wproj, bproj):
    global _NC_CACHE, LAST_RESULTS
    x = np.asarray(x, dtype=np.float32)
    Wqkv = np.asarray(Wqkv, dtype=np.float32)
    bqkv = np.asarray(bqkv, dtype=np.float32)
    Wproj = np.asarray(Wproj, dtype=np.float32)
    bproj = np.asarray(bproj, dtype=np.float32)

    if _NC_CACHE is None:
        _NC_CACHE = _build_program()
    nc = _NC_CACHE

    in_maps = _make_in_maps(x, Wqkv, bqkv, Wproj, bproj)
    res = run_bass_kernel_spmd(nc, in_maps, core_ids=list(range(N_CORES)))
    LAST_RESULTS = res

    out = np.empty((B, L, D), dtype=np.float32)
    for b in range(B):
        acc = res.results[4 * b]["yT"].astype(np.float32)
        for g in range(1, 4):
            acc = acc + res.results[4 * b + g]["yT"]
        # yT tiles indexed s*N_DK+m, each [128 dcols, 512 tokens]
        yT = acc.reshape(NS, N_DK, 128, 512).transpose(1, 2, 0, 3).reshape(D, L)
        out[b] = yT.T
    return out


# revision 22
# speedup vs baseline: 1.2513x; 1.0386x over previous
"""Causal self-attention (B=2, L=2048, D=1024, H=16) on 8 trn2 NeuronCores.

Sharding: core c = 4*b + g handles batch b and head group g (4 heads).
Per core: QKV projection for its heads' weight columns (tensor-parallel),
flash-style causal attention for its 4 heads, and a partial output
projection over its 256 head-dims (row-parallel).  The host sums the 4
partial projections per batch and adds bproj.

Device layout: activations kept transposed (feature-major) throughout:
  xT [D, L] (bf16, DMA'd as contiguous [128,512] tiles) -> Q^T packed per
  head-pair [128, L]; K^T stored as 4 zero-padded [128, L] bf16 tensors
  (head h occupies its 64 partition rows, the rest are zero) so score
  matmuls run at K=128 contraction; V in natural [L, 4*65] bf16 layout --
  col 64 of each head group is 1.0 so the attention row-sum Z rides along
  in the AV matmul; S^T tiles [k, q] so softmax needs no transposes; both
  heads of a pair share one [128,1024] two-bank psum tile so off-diagonal
  exp runs as a single ACT op; causal diagonal handled by a 0/1 bf16
  triangular MULTIPLY on the exp output (GpSimd, SBUF-only engine);
  softmax 1/Z computed once per block on a [2,512] tile (both heads) and
  broadcast through a selector matmul.  All matmuls bf16, fp32 psum.
Input/output DMA uses tile-contiguous DRAM layouts and is issued
round-robin across engine queues (the Sync queue alone serializes at
~0.6us per descriptor).
"""

import sys
import types

import numpy as np


def _install_ntff_shim():
    """The container's antenv stub lacks axon_hooks; recreate it so
    run_bass_kernel_spmd(trace=True) can reach the NTFF profiler."""
    if "antenv.axon_hooks" in sys.modules:
        return
    try:
        import antenv
        from trn_agent_boot.trn_boot import _ntff_profile_via_ctypes
    except Exception:
        return
    mod = types.ModuleType("antenv.axon_hooks")
    hook = _ntff_profile_via_ctypes("/opt/axon/libaxon_pjrt.so")
    mod.get_axon_ntff_profile_hook = lambda: hook
    mod.set_axon_ntff_profile_hook = lambda h: None
    sys.modules["antenv.axon_hooks"] = mod
    antenv.axon_hooks = mod


_install_ntff_shim()

import ml_dtypes  # noqa: E402

import concourse.bass as bass  # noqa: E402
import concourse.mybir as mybir  # noqa: E402
import concourse.tile as tile  # noqa: E402
from concourse.bass_utils import run_bass_kernel_spmd  # noqa: E402
from concourse.vector_clock import ScopedClock, VectorClock  # noqa: E402

B, L, D, H = 2, 2048, 1024, 16
HD = D // H  # 64
N_CORES = 8
HPC = 4  # heads per core
CD = HPC * HD  # 256 head-dims per core
VW = HPC * (HD + 1)  # 260 interleaved V columns (64 vals + ones col per head)
SCALE = HD**-0.5  # 0.125
F32 = mybir.dt.float32
B16 = mybir.dt.bfloat16
NPB16 = ml_dtypes.bfloat16

KT = L // 128  # 16 k-tiles of 128 keys
NS = L // 512  # 4 query chunks of 512
N_DK = D // 128  # 8 feature k-tiles
AV_DELAY = 8  # AV matmul issues this many (k,h)-steps behind its exp


class _TileContext(tile.TileContext):
    """Split exit-drain sem waits to 1 per drain; this walrus build's
    CTRL codegen rejects drains with 2+ sync waits."""

    def _drain_and_barrier(self, tick_clock, wait_clock):
        g = tick_clock.global_clock
        n = len(g)
        procs = [i for i in range(n) if g[i] > 0]
        for p in procs:
            vec = [g[i] if i == p else 0 for i in range(n)]
            d = self.nc.sync.drain()
            wait_clock.add_sem_waits(d.ins, ScopedClock({None: VectorClock(vec)}))
        self.nc.all_engine_barrier()
        popped = self.nc._tile_sem_poison_stack.pop()
        assert popped is self._sem_poison
        self.nc.clear_and_free_semaphores(list(self.sems.allocated().values()))
        self.nc.all_engine_barrier()


def _split_multi_waits(nc):
    """This walrus build's codegen accepts only ONE sync wait per
    instruction; hoist extra waits onto preceding same-engine NOPs."""
    for f in nc.m.functions:
        for blk in f.blocks:
            orig = list(blk.instructions)
            expanded = []
            changed = False
            for ins in orig:
                si = ins.sync_info
                if si is not None and si.on_wait is not None and len(si.on_wait) > 1:
                    changed = True
                    waits = list(si.on_wait)
                    eng = nc.engines[ins.engine]
                    for w in waits[:-1]:
                        nop = eng.nop(nofuse=True).ins
                        # eng.nop() auto-appends to the CURRENT bb; pull it
                        # out -- we re-insert it before `ins` in ins's bb.
                        nc.cur_bb.bb.instructions.remove(nop)
                        nop.sync_info = mybir.SyncInfo(on_wait=[w], on_update=[])
                        expanded.append(nop)
                    ins.sync_info = mybir.SyncInfo(
                        on_wait=[waits[-1]], on_update=list(si.on_update or [])
                    )
                expanded.append(ins)
            if changed:
                il = blk.instructions
                for ins in list(il):
                    il.remove(ins)
                for ins in expanded:
                    il.append(ins)


def _build_program():
    nc = bass.Bass()
    # tile-contiguous DRAM layouts: each DMA descriptor moves one
    # contiguous [128, C] block (large linear packets, no row striding)
    xT_d = nc.dram_tensor("xT", [N_DK * NS, 128, 512], B16, kind="ExternalInput").ap()
    wqkv_d = nc.dram_tensor(
        "wqkv", [N_DK, 128, 2 * CD + VW], B16, kind="ExternalInput"
    ).ap()
    bqk_d = nc.dram_tensor("bqk", [128, 4], F32, kind="ExternalInput").ap()
    bv_d = nc.dram_tensor("bv", [1, VW], B16, kind="ExternalInput").ap()
    wproj_d = nc.dram_tensor("wproj", [2, 128, D], B16, kind="ExternalInput").ap()
    bproj_d = nc.dram_tensor("bproj", [128, N_DK], F32, kind="ExternalInput").ap()
    onesr_d = nc.dram_tensor("onesr", [1, 512], B16, kind="ExternalInput").ap()
    tri_d = nc.dram_tensor("trimask", [128, 128], B16, kind="ExternalInput").ap()
    sel_d = nc.dram_tensor("sel", [2, 128], B16, kind="ExternalInput").ap()
    selo_d = nc.dram_tensor("selo", [128, 128], B16, kind="ExternalInput").ap()
    zer_d = nc.dram_tensor("zer", [64, L], B16, kind="ExternalInput").ap()
    yT_d = nc.dram_tensor("yT", [NS * N_DK, 128, 512], B16, kind="ExternalOutput").ap()

    mm = nc.tensor.matmul

    with _TileContext(nc) as tc, tc.tile_pool(name="sb", bufs=1) as sb, tc.tile_pool(
        name="ps", bufs=1, space="PSUM"
    ) as ps:
        dma_engs = [nc.sync, nc.gpsimd, nc.scalar]
        _rr = [0]

        def dma(out, in_, engs=None):
            pool = engs if engs is not None else dma_engs
            e = pool[_rr[0] % len(pool)]
            _rr[0] += 1
            e.dma_start(out=out, in_=in_)

        # ---- persistent SBUF tensors; DMA issue spread across engine
        # queues (sync alone serializes at ~0.6us/descriptor) ----
        wqkv = [
            sb.tile([128, 2 * CD + VW], B16, tag=f"wqkv{k}", bufs=1, name=f"wqkv{k}")
            for k in range(N_DK)
        ]
        xTc = [
            [
                sb.tile([128, 512], B16, tag=f"xT{k}_{s}", bufs=1, name=f"xT{k}_{s}")
                for s in range(NS)
            ]
            for k in range(N_DK)
        ]
        # first wave: everything the s=0 QKV accumulation chains need,
        # split into half-tile descriptors to spread across more hw queues
        for k in range(N_DK):
            dma(wqkv[k][0:64, :], wqkv_d[k][0:64, :])
            dma(wqkv[k][64:128, :], wqkv_d[k][64:128, :])
            dma(xTc[k][0][0:64, :], xT_d[k * NS + 0][0:64, :])
            dma(xTc[k][0][64:128, :], xT_d[k * NS + 0][64:128, :])
        # ---- constants (host-supplied; memset/affine_select of f32r
        # fail this walrus build's ISA checks) ----
        ones = sb.tile([1, 512], B16, tag="ones", bufs=1)
        nc.sync.dma_start(out=ones[:], in_=onesr_d[:])
        tri = sb.tile([128, 128], B16, tag="tri", bufs=1)
        nc.sync.dma_start(out=tri[:], in_=tri_d[:])
        sel = sb.tile([2, 128], B16, tag="sel", bufs=1)
        nc.sync.dma_start(out=sel[:], in_=sel_d[:])
        selo = sb.tile([128, 128], B16, tag="selo", bufs=1)
        nc.sync.dma_start(out=selo[:], in_=selo_d[:])
        bqk = sb.tile([128, 4], F32, tag="bqk", bufs=1)
        nc.sync.dma_start(out=bqk[:], in_=bqk_d[:])
        bv = sb.tile([1, VW], B16, tag="bv", bufs=1)
        nc.sync.dma_start(out=bv[:], in_=bv_d[:])
        bproj = sb.tile([128, N_DK], F32, tag="bproj", bufs=1)
        nc.sync.dma_start(out=bproj[:], in_=bproj_d[:])

        late_engs = [nc.sync, nc.gpsimd, nc.scalar]
        for s in range(1, NS):
            for k in range(N_DK):
                dma(xTc[k][s][:], xT_d[k * NS + s], engs=late_engs)
        wproj = []
        for kt in range(2):
            t = sb.tile([128, D], B16, tag=f"wproj{kt}", bufs=1)
            dma(t[:], wproj_d[kt], engs=late_engs)
            wproj.append(t)
        # Q^T packed per head pair (rows 0-63 = head 2p, 64-127 = head 2p+1)
        qT = [sb.tile([128, L], B16, tag=f"qT{p}", bufs=1, name=f"qT{p}") for p in range(2)]
        # K^T zero-padded per head: kz[p][h] has head 2p+h in its own 64
        # rows, zeros elsewhere -> K=128 score matmuls pick out one head.
        kz = [
            [
                sb.tile([128, L], B16, tag=f"kz{p}{h}", bufs=1, name=f"kz{p}{h}")
                for h in range(2)
            ]
            for p in range(2)
        ]
        for p in range(2):
            dma(kz[p][0][64:128, :], zer_d[:], engs=late_engs)
            dma(kz[p][1][0:64, :], zer_d[:], engs=late_engs)
        # V natural layout, 16 token tiles of [128, 4*65]; col 64 of each
        # head group = 1.0 (from interleaved W zero-cols + bias ones row)
        vsb = [sb.tile([128, VW], B16, tag=f"v{t}", bufs=1, name=f"v{t}") for t in range(KT)]
        attnT = [sb.tile([128, L], B16, tag=f"attnT{k}", bufs=1, name=f"attnT{k}") for k in range(2)]
        # Z batch tiles: 4 Z rows per group (partitions 0/32/64/96) so ONE
        # [128,512] DVE reciprocal serves two attention blocks (DVE op cost
        # is free-size * cycles regardless of partition count)
        zbs = [sb.tile([128, 512], F32, tag=f"zb{i}", bufs=1, name=f"zb{i}") for i in range(2)]
        rzbs = [sb.tile([128, 512], B16, tag=f"rzb{i}", bufs=1, name=f"rzb{i}") for i in range(2)]
        for t in zbs:
            nc.gpsimd.memset(t[:], 1.0)

        # ============== interleaved QKV / attention / proj ==============
        # Emission order per s-chunk: QKV chunk s, then proj chunk s-2,
        # then attention blocks (0,s),(1,s).  Tile deps keep it correct;
        # interleaving keeps the PE queue full (no p-state resets) and
        # spreads the ACT exp work across the whole kernel.
        TSADD = mybir.AluOpType.add

        def emit_qkv_chunk(s):
            # Q/K part: out[wcol, token] = wqkv[:, m-tile].T @ xT
            for m in range(4):
                p_qk = ps.tile([128, 512], F32, tag="mm", bufs=2)
                for k in range(N_DK):
                    mm(
                        p_qk[:],
                        wqkv[k][:, 128 * m : 128 * (m + 1)],
                        xTc[k][s][:],
                        start=(k == 0),
                        stop=(k == N_DK - 1),
                    )
                # copy to SBUF (bf16) with per-partition (wcol) bias add
                cs = slice(512 * s, 512 * (s + 1))
                if m < 2:
                    nc.vector.tensor_scalar(
                        qT[m][:, cs], p_qk[:], bqk[:, m : m + 1], None, op0=TSADD
                    )
                else:
                    p = m - 2
                    nc.vector.tensor_scalar(
                        kz[p][0][0:64, cs], p_qk[0:64, :],
                        bqk[0:64, m : m + 1], None, op0=TSADD,
                    )
                    nc.vector.tensor_scalar(
                        kz[p][1][64:128, cs], p_qk[64:128, :],
                        bqk[64:128, m : m + 1], None, op0=TSADD,
                    )
            # V part: out[token, vcol] = xT[:, tt].T @ wv_interleaved
            for j in range(4):
                t = 4 * s + j
                p_v = ps.tile([128, VW], F32, tag="mm", bufs=2)
                for k in range(N_DK):
                    mm(
                        p_v[:],
                        xTc[k][s][:, 128 * j : 128 * (j + 1)],
                        wqkv[k][:, 2 * CD : 2 * CD + VW],
                        start=(k == 0),
                        stop=False,
                    )
                mm(p_v[:], ones[0:1, 0:128], bv[:], start=False, stop=True)
                nc.scalar.copy(vsb[t][:], p_v[:])

        def emit_flush(av, pair, s):
            # block end: pull Z rows into the group tile (partition
            # 64*pair+32*h) and the unnormalized AV rows out of psum
            zb = zbs[s % 2]
            uns = []
            for h in range(2):
                r = 64 * pair + 32 * h
                nc.vector.tensor_copy(zb[r : r + 1, :], av[h][64:65, :])
                un = sb.tile([64, 512], F32, tag="un", bufs=6, name="un")
                nc.vector.tensor_copy(un[:], av[h][0:64, :])
                uns.append(un)
            return uns

        def emit_group_recip(s):
            # one reciprocal for all 4 Z rows of query-chunk s
            with nc.allow_low_precision(reason="1/Z in bf16"):
                nc.vector.reciprocal(rzbs[s % 2][:], zbs[s % 2][:])

        def emit_norm(pair, q0, s, uns):
            rzb = rzbs[s % 2]
            for h in range(2):
                b0 = 64 * pair
                # broadcast 1/Z_h (row 64*pair+32*h of rzb) via a K=64
                # selector matmul; psum held only for the short DVE multiply
                bc_ps = ps.tile([64, 512], F32, tag="mm", bufs=2, name="bc_ps")
                mm(
                    bc_ps[:],
                    selo[b0 : b0 + 64, 64 * h : 64 * h + 64],
                    rzb[b0 : b0 + 64, :],
                    start=True,
                    stop=True,
                )
                if h == 0:
                    nc.vector.tensor_tensor(
                        attnT[pair][0:64, q0 : q0 + 512],
                        uns[h][:],
                        bc_ps[:],
                        op=mybir.AluOpType.mult,
                    )
                else:
                    tmp = sb.tile([64, 512], B16, tag="ntmp", bufs=2, name="tmp")
                    nc.vector.tensor_tensor(
                        tmp[:], uns[h][:], bc_ps[:], op=mybir.AluOpType.mult
                    )
                    nc.gpsimd.dma_start(
                        out=attnT[pair][64:128, q0 : q0 + 512], in_=tmp[:]
                    )

        def emit_proj_chunk(u):
            for m in range(N_DK):
                p_y = ps.tile([128, 512], F32, tag="mm", bufs=2)
                for kt in range(2):
                    mm(
                        p_y[:],
                        wproj[kt][:, 128 * m : 128 * (m + 1)],
                        attnT[kt][:, 512 * u : 512 * (u + 1)],
                        start=(kt == 0),
                        stop=(kt == 1),
                    )
                y_sb = sb.tile([128, 512], B16, tag="ysb", bufs=6)
                nc.vector.tensor_scalar(
                    y_sb[:], p_y[:], bproj[:, m : m + 1], None, op0=TSADD
                )
                nc.gpsimd.dma_start(out=yT_d[u * N_DK + m], in_=y_sb[:])

        pending = []  # (block_id, mm_args, mm_kwargs)
        state = dict(fin_prev=None, uns={}, norms_due=None)

        def emit_attn_block(bid, pair, s):
            q0 = 512 * s
            n_k = 4 * s + 4
            av = [
                ps.tile([65, 512], F32, tag=f"av{h}", bufs=1, name=f"av{h}")
                for h in range(2)
            ]
            for k in range(n_k):
                k0 = 128 * k
                diag_t = k - 4 * s
                lo = 128 * diag_t if diag_t >= 0 else 0
                # both heads' scores into one 2-bank psum tile; halves at
                # column offsets 0 / 512 so the off-diagonal exp covers both
                s_ps = ps.tile([128, 1024], F32, tag="st", bufs=2)
                pt = sb.tile([128, 1024], B16, tag="pt", bufs=AV_DELAY // 2 + 2)
                for h in range(2):
                    c0 = 512 * h
                    mm(
                        s_ps[:, c0 + lo : c0 + 512],
                        kz[pair][h][:, k0 : k0 + 128],
                        qT[pair][:, q0 + lo : q0 + 512],
                        start=True,
                        stop=True,
                    )
                if diag_t >= 0:
                    for h in range(2):
                        c0 = 512 * h
                        nc.scalar.activation(
                            pt[:, c0 + lo : c0 + 512],
                            s_ps[:, c0 + lo : c0 + 512],
                            mybir.ActivationFunctionType.Exp,
                            scale=SCALE,
                        )
                    # causal mask: multiply the diagonal 128x128 block of
                    # exp(S^T) by a 0/1 lower-triangle (GpSimd, sbuf-only)
                    for h in range(2):
                        c0 = 512 * h
                        nc.gpsimd.tensor_tensor(
                            pt[:, c0 + lo : c0 + lo + 128],
                            pt[:, c0 + lo : c0 + lo + 128],
                            tri[:],
                            op=mybir.AluOpType.mult,
                        )
                else:
                    nc.scalar.activation(
                        pt[:],
                        s_ps[:],
                        mybir.ActivationFunctionType.Exp,
                        scale=SCALE,
                    )
                for h in range(2):
                    hg = 2 * pair + h
                    c0 = 512 * h
                    pending.append(
                        (
                            bid,
                            (
                                av[h][0:65, lo:512],
                                vsb[k][:, 65 * hg : 65 * hg + 65],
                                pt[:, c0 + lo : c0 + 512],
                            ),
                            dict(
                                start=(k == 0),
                                stop=(k == n_k - 1),
                                skip_group_check=True,
                            ),
                        )
                    )
                    while len(pending) > AV_DELAY:
                        _, a, kw = pending.pop(0)
                        mm(*a, **kw)
                if k == 1 and state["fin_prev"] is not None:
                    # flush the previous block's tail AVs and free its av
                    # psum; once both blocks of a query chunk are flushed,
                    # run the shared reciprocal + both normalizes
                    pbid = state["fin_prev"][0]
                    while pending and pending[0][0] == pbid:
                        _, a, kw = pending.pop(0)
                        mm(*a, **kw)
                    _, ppair, ps_, pav = state["fin_prev"]
                    uns = emit_flush(pav, ppair, ps_)
                    state["uns"][(ppair, ps_)] = uns
                    if ppair == 1:
                        emit_group_recip(ps_)
                        state["norms_due"] = ps_
                    state["fin_prev"] = None
                if k == 5 and state["norms_due"] is not None:
                    # norms deferred a few PE-steps past the reciprocal so
                    # the psum broadcast tiles never wait on the DVE
                    ns_ = state["norms_due"]
                    state["norms_due"] = None
                    for npair in range(2):
                        emit_norm(
                            npair, 512 * ns_, ns_,
                            state["uns"].pop((npair, ns_)),
                        )
            state["fin_prev"] = (bid, pair, s, av)

        bid = 0
        for s in range(NS):
            emit_qkv_chunk(s)
            emit_attn_block(bid, 0, s)
            bid += 1
            if s >= 1:
                emit_proj_chunk(s - 1)
            emit_attn_block(bid, 1, s)
            bid += 1
        while pending:
            _, a, kw = pending.pop(0)
            mm(*a, **kw)
        _, ppair, ps_, pav = state["fin_prev"]
        uns = emit_flush(pav, ppair, ps_)
        state["uns"][(ppair, ps_)] = uns
        emit_group_recip(ps_)
        for npair in range(2):
            emit_norm(npair, 512 * ps_, ps_, state["uns"].pop((npair, ps_)))
        emit_proj_chunk(3)
    _split_multi_waits(nc)
    return nc


_NC_CACHE = None
LAST_RESULTS = None

_ONESR = np.ones((1, 512), dtype=NPB16)
_ZER = np.zeros((64, L), dtype=NPB16)
_I, _J = np.meshgrid(np.arange(128), np.arange(128), indexing="ij")
_TRI = (_J >= _I).astype(NPB16)  # 1.0 keep / 0.0 mask on the S^T diag block
_SELO = np.zeros((128, 128), dtype=NPB16)
_SELO[0, 0:64] = 1.0
_SELO[64, 0:64] = 1.0
_SELO[32, 64:128] = 1.0
_SELO[96, 64:128] = 1.0
_SEL = np.zeros((2, 128), dtype=NPB16)
_SEL[0, 0:64] = 1.0
_SEL[1, 64:128] = 1.0


def _make_in_maps(x, Wqkv, bqkv, Wproj, bproj):
    in_maps = []
    for c in range(N_CORES):
        b, g = divmod(c, 4)
        qc = slice(CD * g, CD * (g + 1))
        wq = Wqkv[:, qc]
        wk = Wqkv[:, D : 2 * D][:, qc]
        wv = Wqkv[:, 2 * D : 3 * D][:, qc]
        bq = bqkv[qc]
        bk = bqkv[D : 2 * D][qc]
        bvv = bqkv[2 * D : 3 * D][qc]
        # V columns interleaved per head: [wv_h (64 cols) | zeros col] so the
        # psum comes out in vsb layout; bv row gets [bv_h | 1.0].
        wv_i = np.zeros((D, VW), dtype=np.float32)
        bv_i = np.zeros((1, VW), dtype=np.float32)
        for h in range(HPC):
            wv_i[:, 65 * h : 65 * h + 64] = wv[:, 64 * h : 64 * h + 64]
            bv_i[0, 65 * h : 65 * h + 64] = bvv[64 * h : 64 * h + 64]
            bv_i[0, 65 * h + 64] = 1.0
        bqk_cols = np.concatenate([bq, bk]).reshape(4, 128).T  # [128, 4]
        xT = x[b].T.astype(NPB16)  # [D, L]
        xT_t = np.ascontiguousarray(
            xT.reshape(N_DK, 128, NS, 512).transpose(0, 2, 1, 3)
        ).reshape(N_DK * NS, 128, 512)
        wqkv_full = np.concatenate([wq, wk, wv_i], axis=1).astype(NPB16)
        wqkv_t = np.ascontiguousarray(wqkv_full.reshape(N_DK, 128, 2 * CD + VW))
        wproj_t = np.ascontiguousarray(
            Wproj[CD * g : CD * (g + 1), :].astype(NPB16).reshape(2, 128, D)
        )
        in_maps.append(
            {
                "xT": xT_t,
                "wqkv": wqkv_t,
                "bqk": np.ascontiguousarray(bqk_cols),
                "bv": bv_i.astype(NPB16),
                "wproj": wproj_t,
                "bproj": np.ascontiguousarray(
                    (bproj if g == 0 else np.zeros_like(bproj)).reshape(N_DK, 128).T
                ),
                "onesr": _ONESR,
                "trimask": _TRI,
                "sel": _SEL,
                "selo": _SELO,
                "zer": _ZER,
            }
        )

    return in_maps


def kernel(x, Wqkv, bqkv, Wproj, bproj):
    global _NC_CACHE, LAST_RESULTS
    x = np.asarray(x, dtype=np.float32)
    Wqkv = np.asarray(Wqkv, dtype=np.float32)
    bqkv = np.asarray(bqkv, dtype=np.float32)
    Wproj = np.asarray(Wproj, dtype=np.float32)
    bproj = np.asarray(bproj, dtype=np.float32)

    if _NC_CACHE is None:
        _NC_CACHE = _build_program()
    nc = _NC_CACHE

    in_maps = _make_in_maps(x, Wqkv, bqkv, Wproj, bproj)
    res = run_bass_kernel_spmd(nc, in_maps, core_ids=list(range(N_CORES)))
    LAST_RESULTS = res

    out = np.empty((B, L, D), dtype=np.float32)
    for b in range(B):
        acc = res.results[4 * b]["yT"].astype(np.float32)
        for g in range(1, 4):
            acc = acc + res.results[4 * b + g]["yT"]
        # yT tiles indexed s*N_DK+m, each [128 dcols, 512 tokens]
        yT = acc.reshape(NS, N_DK, 128, 512).transpose(1, 2, 0, 3).reshape(D, L)
        out[b] = yT.T
    return out


# revision 27
# speedup vs baseline: 1.2553x; 1.0032x over previous
"""Causal self-attention (B=2, L=2048, D=1024, H=16) on 8 trn2 NeuronCores.

Sharding: core c = 4*b + g handles batch b and head group g (4 heads).
Per core: QKV projection for its heads' weight columns (tensor-parallel),
flash-style causal attention for its 4 heads, and a partial output
projection over its 256 head-dims (row-parallel).  The host sums the 4
partial projections per batch and adds bproj.

Device layout: activations kept transposed (feature-major) throughout:
  xT [D, L] (bf16, DMA'd as contiguous [128,512] tiles) -> Q^T packed per
  head-pair [128, L]; K^T stored as 4 zero-padded [128, L] bf16 tensors
  (head h occupies its 64 partition rows, the rest are zero) so score
  matmuls run at K=128 contraction; V in natural [L, 4*65] bf16 layout --
  col 64 of each head group is 1.0 so the attention row-sum Z rides along
  in the AV matmul; S^T tiles [k, q] so softmax needs no transposes; both
  heads of a pair share one [128,1024] two-bank psum tile so off-diagonal
  exp runs as a single ACT op; causal diagonal handled by a 0/1 bf16
  triangular MULTIPLY on the exp output (GpSimd, SBUF-only engine);
  softmax 1/Z computed once per block on a [2,512] tile (both heads) and
  broadcast through a selector matmul.  All matmuls bf16, fp32 psum.
Input/output DMA uses tile-contiguous DRAM layouts and is issued
round-robin across engine queues (the Sync queue alone serializes at
~0.6us per descriptor).
"""

import sys
import types

import numpy as np


def _install_ntff_shim():
    """The container's antenv stub lacks axon_hooks; recreate it so
    run_bass_kernel_spmd(trace=True) can reach the NTFF profiler."""
    if "antenv.axon_hooks" in sys.modules:
        return
    try:
        import antenv
        from trn_agent_boot.trn_boot import _ntff_profile_via_ctypes
    except Exception:
        return
    mod = types.ModuleType("antenv.axon_hooks")
    hook = _ntff_profile_via_ctypes("/opt/axon/libaxon_pjrt.so")
    mod.get_axon_ntff_profile_hook = lambda: hook
    mod.set_axon_ntff_profile_hook = lambda h: None
    sys.modules["antenv.axon_hooks"] = mod
    antenv.axon_hooks = mod


_install_ntff_shim()

import ml_dtypes  # noqa: E402

import concourse.bass as bass  # noqa: E402
import concourse.mybir as mybir  # noqa: E402
import concourse.tile as tile  # noqa: E402
from concourse.bass_utils import run_bass_kernel_spmd  # noqa: E402
from concourse.vector_clock import ScopedClock, VectorClock  # noqa: E402

B, L, D, H = 2, 2048, 1024, 16
HD = D // H  # 64
N_CORES = 8
HPC = 4  # heads per core
CD = HPC * HD  # 256 head-dims per core
VW = HPC * (HD + 1)  # 260 interleaved V columns (64 vals + ones col per head)
SCALE = HD**-0.5  # 0.125
F32 = mybir.dt.float32
B16 = mybir.dt.bfloat16
FP8 = mybir.dt.float8e4
NPB16 = ml_dtypes.bfloat16

KT = L // 128  # 16 k-tiles of 128 keys
NS = L // 512  # 4 query chunks of 512
N_DK = D // 128  # 8 feature k-tiles
AV_DELAY = 8  # AV matmul issues this many (k,h)-steps behind its exp


class _TileContext(tile.TileContext):
    """Split exit-drain sem waits to 1 per drain; this walrus build's
    CTRL codegen rejects drains with 2+ sync waits."""

    def _drain_and_barrier(self, tick_clock, wait_clock):
        g = tick_clock.global_clock
        n = len(g)
        procs = [i for i in range(n) if g[i] > 0]
        for p in procs:
            vec = [g[i] if i == p else 0 for i in range(n)]
            d = self.nc.sync.drain()
            wait_clock.add_sem_waits(d.ins, ScopedClock({None: VectorClock(vec)}))
        self.nc.all_engine_barrier()
        popped = self.nc._tile_sem_poison_stack.pop()
        assert popped is self._sem_poison
        self.nc.clear_and_free_semaphores(list(self.sems.allocated().values()))
        self.nc.all_engine_barrier()


def _split_multi_waits(nc):
    """This walrus build's codegen accepts only ONE sync wait per
    instruction; hoist extra waits onto preceding same-engine NOPs."""
    for f in nc.m.functions:
        for blk in f.blocks:
            orig = list(blk.instructions)
            expanded = []
            changed = False
            for ins in orig:
                si = ins.sync_info
                if si is not None and si.on_wait is not None and len(si.on_wait) > 1:
                    changed = True
                    waits = list(si.on_wait)
                    eng = nc.engines[ins.engine]
                    for w in waits[:-1]:
                        nop = eng.nop(nofuse=True).ins
                        # eng.nop() auto-appends to the CURRENT bb; pull it
                        # out -- we re-insert it before `ins` in ins's bb.
                        nc.cur_bb.bb.instructions.remove(nop)
                        nop.sync_info = mybir.SyncInfo(on_wait=[w], on_update=[])
                        expanded.append(nop)
                    ins.sync_info = mybir.SyncInfo(
                        on_wait=[waits[-1]], on_update=list(si.on_update or [])
                    )
                expanded.append(ins)
            if changed:
                il = blk.instructions
                for ins in list(il):
                    il.remove(ins)
                for ins in expanded:
                    il.append(ins)


def _build_program():
    nc = bass.Bass()
    # tile-contiguous DRAM layouts: each DMA descriptor moves one
    # contiguous [128, C] block (large linear packets, no row striding)
    xT_d = nc.dram_tensor("xT", [N_DK * NS, 128, 512], B16, kind="ExternalInput").ap()
    wqkv_d = nc.dram_tensor(
        "wqkv", [N_DK, 128, 2 * CD + VW], B16, kind="ExternalInput"
    ).ap()
    bqk_d = nc.dram_tensor("bqk", [128, 4], F32, kind="ExternalInput").ap()
    bv_d = nc.dram_tensor("bv", [1, VW], B16, kind="ExternalInput").ap()
    wproj_d = nc.dram_tensor("wproj", [2, 128, D], B16, kind="ExternalInput").ap()
    bproj_d = nc.dram_tensor("bproj", [128, N_DK], F32, kind="ExternalInput").ap()
    onesr_d = nc.dram_tensor("onesr", [1, 512], B16, kind="ExternalInput").ap()
    tri_d = nc.dram_tensor("trimask", [128, 128], B16, kind="ExternalInput").ap()
    sel_d = nc.dram_tensor("sel", [2, 128], B16, kind="ExternalInput").ap()
    selo_d = nc.dram_tensor("selo", [128, 128], B16, kind="ExternalInput").ap()
    zer_d = nc.dram_tensor("zer", [64, L], B16, kind="ExternalInput").ap()
    yT_d = nc.dram_tensor("yT", [NS * N_DK, 128, 512], B16, kind="ExternalOutput").ap()

    mm = nc.tensor.matmul

    with _TileContext(nc) as tc, tc.tile_pool(name="sb", bufs=1) as sb, tc.tile_pool(
        name="ps", bufs=1, space="PSUM"
    ) as ps:
        dma_engs = [nc.sync, nc.gpsimd, nc.scalar]
        _rr = [0]

        def dma(out, in_, engs=None):
            pool = engs if engs is not None else dma_engs
            e = pool[_rr[0] % len(pool)]
            _rr[0] += 1
            e.dma_start(out=out, in_=in_)

        # ---- persistent SBUF tensors; DMA issue spread across engine
        # queues (sync alone serializes at ~0.6us/descriptor) ----
        wqkv = [
            sb.tile([128, 2 * CD + VW], B16, tag=f"wqkv{k}", bufs=1, name=f"wqkv{k}")
            for k in range(N_DK)
        ]
        xTc = [
            [
                sb.tile([128, 512], B16, tag=f"xT{k}_{s}", bufs=1, name=f"xT{k}_{s}")
                for s in range(NS)
            ]
            for k in range(N_DK)
        ]
        # first wave: everything the s=0 QKV accumulation chains need,
        # split into half-tile descriptors to spread across more hw queues
        for k in range(N_DK):
            dma(wqkv[k][0:64, :], wqkv_d[k][0:64, :])
            dma(wqkv[k][64:128, :], wqkv_d[k][64:128, :])
            dma(xTc[k][0][0:64, :], xT_d[k * NS + 0][0:64, :])
            dma(xTc[k][0][64:128, :], xT_d[k * NS + 0][64:128, :])
        # ---- constants (host-supplied; memset/affine_select of f32r
        # fail this walrus build's ISA checks) ----
        ones = sb.tile([1, 512], B16, tag="ones", bufs=1)
        nc.sync.dma_start(out=ones[:], in_=onesr_d[:])
        tri = sb.tile([128, 128], B16, tag="tri", bufs=1)
        nc.sync.dma_start(out=tri[:], in_=tri_d[:])
        sel = sb.tile([2, 128], B16, tag="sel", bufs=1)
        nc.sync.dma_start(out=sel[:], in_=sel_d[:])
        selo = sb.tile([128, 128], B16, tag="selo", bufs=1)
        nc.sync.dma_start(out=selo[:], in_=selo_d[:])
        bqk = sb.tile([128, 4], F32, tag="bqk", bufs=1)
        nc.sync.dma_start(out=bqk[:], in_=bqk_d[:])
        bv = sb.tile([1, VW], B16, tag="bv", bufs=1)
        nc.sync.dma_start(out=bv[:], in_=bv_d[:])
        bproj = sb.tile([128, N_DK], F32, tag="bproj", bufs=1)
        nc.sync.dma_start(out=bproj[:], in_=bproj_d[:])

        late_engs = [nc.sync, nc.gpsimd, nc.scalar]
        for s in range(1, NS):
            for k in range(N_DK):
                dma(xTc[k][s][:], xT_d[k * NS + s], engs=late_engs)
        wproj = []
        for kt in range(2):
            t = sb.tile([128, D], B16, tag=f"wproj{kt}", bufs=1)
            dma(t[:], wproj_d[kt], engs=late_engs)
            wproj.append(t)
        # Q^T packed per head pair (rows 0-63 = head 2p, 64-127 = head 2p+1)
        qT = [sb.tile([128, L], B16, tag=f"qT{p}", bufs=1, name=f"qT{p}") for p in range(2)]
        # K^T zero-padded per head: kz[p][h] has head 2p+h in its own 64
        # rows, zeros elsewhere -> K=128 score matmuls pick out one head.
        kz = [
            [
                sb.tile([128, L], B16, tag=f"kz{p}{h}", bufs=1, name=f"kz{p}{h}")
                for h in range(2)
            ]
            for p in range(2)
        ]
        for p in range(2):
            dma(kz[p][0][64:128, :], zer_d[:], engs=late_engs)
            dma(kz[p][1][0:64, :], zer_d[:], engs=late_engs)
        # V natural layout, 16 token tiles of [128, 4*65]; col 64 of each
        # head group = 1.0 (from interleaved W zero-cols + bias ones row)
        vsb = [sb.tile([128, VW], B16, tag=f"v{t}", bufs=1, name=f"v{t}") for t in range(KT)]
        attnT = [sb.tile([128, L], B16, tag=f"attnT{k}", bufs=1, name=f"attnT{k}") for k in range(2)]
        # Z batch tiles: 4 Z rows per group (partitions 0/32/64/96) so ONE
        # [128,512] DVE reciprocal serves two attention blocks (DVE op cost
        # is free-size * cycles regardless of partition count)
        zbs = [sb.tile([128, 512], F32, tag=f"zb{i}", bufs=1, name=f"zb{i}") for i in range(2)]
        rzbs = [sb.tile([128, 512], B16, tag=f"rzb{i}", bufs=1, name=f"rzb{i}") for i in range(2)]
        for t in zbs:
            nc.gpsimd.memset(t[:], 1.0)

        # ============== interleaved QKV / attention / proj ==============
        # Emission order per s-chunk: QKV chunk s, then proj chunk s-2,
        # then attention blocks (0,s),(1,s).  Tile deps keep it correct;
        # interleaving keeps the PE queue full (no p-state resets) and
        # spreads the ACT exp work across the whole kernel.
        TSADD = mybir.AluOpType.add

        def emit_qkv_chunk(s):
            # Q/K part: out[wcol, token] = wqkv[:, m-tile].T @ xT
            for m in range(4):
                p_qk = ps.tile([128, 512], F32, tag="mm", bufs=2)
                for k in range(N_DK):
                    mm(
                        p_qk[:],
                        wqkv[k][:, 128 * m : 128 * (m + 1)],
                        xTc[k][s][:],
                        start=(k == 0),
                        stop=(k == N_DK - 1),
                    )
                # copy to SBUF (bf16) with per-partition (wcol) bias add
                cs = slice(512 * s, 512 * (s + 1))
                if m < 2:
                    nc.vector.tensor_scalar(
                        qT[m][:, cs], p_qk[:], bqk[:, m : m + 1], None, op0=TSADD
                    )
                else:
                    p = m - 2
                    nc.vector.tensor_scalar(
                        kz[p][0][0:64, cs], p_qk[0:64, :],
                        bqk[0:64, m : m + 1], None, op0=TSADD,
                    )
                    nc.vector.tensor_scalar(
                        kz[p][1][64:128, cs], p_qk[64:128, :],
                        bqk[64:128, m : m + 1], None, op0=TSADD,
                    )
            # V part: out[token, vcol] = xT[:, tt].T @ wv_interleaved
            for j in range(4):
                t = 4 * s + j
                p_v = ps.tile([128, VW], F32, tag="mm", bufs=2)
                for k in range(N_DK):
                    mm(
                        p_v[:],
                        xTc[k][s][:, 128 * j : 128 * (j + 1)],
                        wqkv[k][:, 2 * CD : 2 * CD + VW],
                        start=(k == 0),
                        stop=False,
                    )
                mm(p_v[:], ones[0:1, 0:128], bv[:], start=False, stop=True)
                nc.scalar.copy(vsb[t][:], p_v[:])

        def emit_flush(av, pair, s):
            # block end: pull Z rows into the group tile (partition
            # 64*pair+32*h) and the unnormalized AV rows out of psum
            zb = zbs[s % 2]
            uns = []
            for h in range(2):
                r = 64 * pair + 32 * h
                nc.vector.tensor_copy(zb[r : r + 1, :], av[h][64:65, :])
                un = sb.tile([64, 512], F32, tag="un", bufs=6, name="un")
                nc.vector.tensor_copy(un[:], av[h][0:64, :])
                uns.append(un)
            return uns

        def emit_group_recip(s):
            # one reciprocal for all 4 Z rows of query-chunk s
            with nc.allow_low_precision(reason="1/Z in bf16"):
                nc.vector.reciprocal(rzbs[s % 2][:], zbs[s % 2][:])

        def emit_norm(pair, q0, s, uns):
            rzb = rzbs[s % 2]
            for h in range(2):
                b0 = 64 * pair
                # broadcast 1/Z_h (row 64*pair+32*h of rzb) via a K=64
                # selector matmul; psum held only for the short DVE multiply
                bc_ps = ps.tile([64, 512], F32, tag="mm", bufs=2, name="bc_ps")
                mm(
                    bc_ps[:],
                    selo[b0 : b0 + 64, 64 * h : 64 * h + 64],
                    rzb[b0 : b0 + 64, :],
                    start=True,
                    stop=True,
                )
                if h == 0:
                    nc.vector.tensor_tensor(
                        attnT[pair][0:64, q0 : q0 + 512],
                        uns[h][:],
                        bc_ps[:],
                        op=mybir.AluOpType.mult,
                    )
                else:
                    tmp = sb.tile([64, 512], B16, tag="ntmp", bufs=2, name="tmp")
                    nc.vector.tensor_tensor(
                        tmp[:], uns[h][:], bc_ps[:], op=mybir.AluOpType.mult
                    )
                    nc.gpsimd.dma_start(
                        out=attnT[pair][64:128, q0 : q0 + 512], in_=tmp[:]
                    )

        def emit_proj_chunk(u):
            for m in range(N_DK):
                p_y = ps.tile([128, 512], F32, tag="mm", bufs=2)
                for kt in range(2):
                    mm(
                        p_y[:],
                        wproj[kt][:, 128 * m : 128 * (m + 1)],
                        attnT[kt][:, 512 * u : 512 * (u + 1)],
                        start=(kt == 0),
                        stop=(kt == 1),
                    )
                y_sb = sb.tile([128, 512], B16, tag="ysb", bufs=6)
                nc.vector.tensor_scalar(
                    y_sb[:], p_y[:], bproj[:, m : m + 1], None, op0=TSADD
                )
                nc.gpsimd.dma_start(out=yT_d[u * N_DK + m], in_=y_sb[:])

        pending = []  # (block_id, mm_args, mm_kwargs)
        state = dict(fin_prev=None, uns={}, norms_due=None)

        def emit_attn_block(bid, pair, s):
            q0 = 512 * s
            n_k = 4 * s + 4
            av = [
                ps.tile([65, 512], F32, tag=f"av{h}", bufs=1, name=f"av{h}")
                for h in range(2)
            ]
            for k in range(n_k):
                k0 = 128 * k
                diag_t = k - 4 * s
                lo = 128 * diag_t if diag_t >= 0 else 0
                # both heads' scores into one 2-bank psum tile; halves at
                # column offsets 0 / 512 so the off-diagonal exp covers both
                s_ps = ps.tile([128, 1024], F32, tag="st", bufs=2)
                for h in range(2):
                    c0 = 512 * h
                    mm(
                        s_ps[:, c0 + lo : c0 + 512],
                        kz[pair][h][:, k0 : k0 + 128],
                        qT[pair][:, q0 + lo : q0 + 512],
                        start=True,
                        stop=True,
                    )
                pt = sb.tile([128, 1024], B16, tag="pt", bufs=AV_DELAY // 2 + 2)
                if diag_t >= 0:
                    for h in range(2):
                        c0 = 512 * h
                        nc.scalar.activation(
                            pt[:, c0 + lo : c0 + 512],
                            s_ps[:, c0 + lo : c0 + 512],
                            mybir.ActivationFunctionType.Exp,
                            scale=SCALE,
                        )
                    # causal mask: multiply the diagonal 128x128 block of
                    # exp(S^T) by a 0/1 lower-triangle (GpSimd, sbuf-only)
                    for h in range(2):
                        c0 = 512 * h
                        nc.gpsimd.tensor_tensor(
                            pt[:, c0 + lo : c0 + lo + 128],
                            pt[:, c0 + lo : c0 + lo + 128],
                            tri[:],
                            op=mybir.AluOpType.mult,
                        )
                else:
                    nc.scalar.activation(
                        pt[:],
                        s_ps[:],
                        mybir.ActivationFunctionType.Exp,
                        scale=SCALE,
                    )
                for h in range(2):
                    hg = 2 * pair + h
                    pending.append(
                        (
                            bid,
                            (
                                av[h][0:65, lo:512],
                                vsb[k][:, 65 * hg : 65 * hg + 65],
                                pt[:, 512 * h + lo : 512 * h + 512],
                            ),
                            dict(
                                start=(k == 0),
                                stop=(k == n_k - 1),
                                skip_group_check=True,
                            ),
                        )
                    )
                    while len(pending) > AV_DELAY:
                        _, a, kw = pending.pop(0)
                        mm(*a, **kw)
                if k == 1 and state["fin_prev"] is not None:
                    # flush the previous block's tail AVs and free its av
                    # psum; once both blocks of a query chunk are flushed,
                    # run the shared reciprocal + both normalizes
                    pbid = state["fin_prev"][0]
                    while pending and pending[0][0] == pbid:
                        _, a, kw = pending.pop(0)
                        mm(*a, **kw)
                    _, ppair, ps_, pav = state["fin_prev"]
                    uns = emit_flush(pav, ppair, ps_)
                    state["uns"][(ppair, ps_)] = uns
                    if ppair == 1:
                        emit_group_recip(ps_)
                        state["norms_due"] = ps_
                    state["fin_prev"] = None
                if k == 5 and state["norms_due"] is not None:
                    # norms deferred a few PE-steps past the reciprocal so
                    # the psum broadcast tiles never wait on the DVE
                    ns_ = state["norms_due"]
                    state["norms_due"] = None
                    for npair in range(2):
                        emit_norm(
                            npair, 512 * ns_, ns_,
                            state["uns"].pop((npair, ns_)),
                        )
            state["fin_prev"] = (bid, pair, s, av)

        bid = 0
        for s in range(NS):
            emit_qkv_chunk(s)
            emit_attn_block(bid, 0, s)
            bid += 1
            if s >= 1:
                emit_proj_chunk(s - 1)
            emit_attn_block(bid, 1, s)
            bid += 1
        while pending:
            _, a, kw = pending.pop(0)
            mm(*a, **kw)
        _, ppair, ps_, pav = state["fin_prev"]
        uns = emit_flush(pav, ppair, ps_)
        state["uns"][(ppair, ps_)] = uns
        emit_group_recip(ps_)
        for npair in range(2):
            emit_norm(npair, 512 * ps_, ps_, state["uns"].pop((npair, ps_)))
        emit_proj_chunk(3)
    _split_multi_waits(nc)
    return nc


_NC_CACHE = None
LAST_RESULTS = None

_ONESR = np.ones((1, 512), dtype=NPB16)
_ZER = np.zeros((64, L), dtype=NPB16)
_I, _J = np.meshgrid(np.arange(128), np.arange(128), indexing="ij")
_TRI = (_J >= _I).astype(NPB16)  # 1.0 keep / 0.0 mask on the S^T diag block
_SELO = np.zeros((128, 128), dtype=NPB16)
_SELO[0, 0:64] = 1.0
_SELO[64, 0:64] = 1.0
_SELO[32, 64:128] = 1.0
_SELO[96, 64:128] = 1.0
_SEL = np.zeros((2, 128), dtype=NPB16)
_SEL[0, 0:64] = 1.0
_SEL[1, 64:128] = 1.0


def _make_in_maps(x, Wqkv, bqkv, Wproj, bproj):
    in_maps = []
    for c in range(N_CORES):
        b, g = divmod(c, 4)
        qc = slice(CD * g, CD * (g + 1))
        wq = Wqkv[:, qc]
        wk = Wqkv[:, D : 2 * D][:, qc]
        wv = Wqkv[:, 2 * D : 3 * D][:, qc]
        bq = bqkv[qc]
        bk = bqkv[D : 2 * D][qc]
        bvv = bqkv[2 * D : 3 * D][qc]
        # V columns interleaved per head: [wv_h (64 cols) | zeros col] so the
        # psum comes out in vsb layout; bv row gets [bv_h | 1.0].
        wv_i = np.zeros((D, VW), dtype=np.float32)
        bv_i = np.zeros((1, VW), dtype=np.float32)
        for h in range(HPC):
            wv_i[:, 65 * h : 65 * h + 64] = wv[:, 64 * h : 64 * h + 64]
            bv_i[0, 65 * h : 65 * h + 64] = bvv[64 * h : 64 * h + 64]
            bv_i[0, 65 * h + 64] = 1.0
        bqk_cols = np.concatenate([bq, bk]).reshape(4, 128).T  # [128, 4]
        xT = x[b].T.astype(NPB16)  # [D, L]
        xT_t = np.ascontiguousarray(
            xT.reshape(N_DK, 128, NS, 512).transpose(0, 2, 1, 3)
        ).reshape(N_DK * NS, 128, 512)
        wqkv_full = np.concatenate([wq, wk, wv_i], axis=1).astype(NPB16)
        wqkv_t = np.ascontiguousarray(wqkv_full.reshape(N_DK, 128, 2 * CD + VW))
        wproj_t = np.ascontiguousarray(
            Wproj[CD * g : CD * (g + 1), :].astype(NPB16).reshape(2, 128, D)
        )
        in_maps.append(
            {
                "xT": xT_t,
                "wqkv": wqkv_t,
                "bqk": np.ascontiguousarray(bqk_cols),
                "bv": bv_i.astype(NPB16),
                "wproj": wproj_t,
                "bproj": np.ascontiguousarray(
                    (bproj if g == 0 else np.zeros_like(bproj)).reshape(N_DK, 128).T
                ),
                "onesr": _ONESR,
                "trimask": _TRI,
                "sel": _SEL,
                "selo": _SELO,
                "zer": _ZER,
            }
        )

    return in_maps


def kernel(x, Wqkv, bqkv, Wproj, bproj):
    global _NC_CACHE, LAST_RESULTS
    x = np.asarray(x, dtype=np.float32)
    Wqkv = np.asarray(Wqkv, dtype=np.float32)
    bqkv = np.asarray(bqkv, dtype=np.float32)
    Wproj = np.asarray(Wproj, dtype=np.float32)
    bproj = np.asarray(bproj, dtype=np.float32)

    if _NC_CACHE is None:
        _NC_CACHE = _build_program()
    nc = _NC_CACHE

    in_maps = _make_in_maps(x, Wqkv, bqkv, Wproj, bproj)
    res = run_bass_kernel_spmd(nc, in_maps, core_ids=list(range(N_CORES)))
    LAST_RESULTS = res

    out = np.empty((B, L, D), dtype=np.float32)
    for b in range(B):
        acc = res.results[4 * b]["yT"].astype(np.float32)
        for g in range(1, 4):
            acc = acc + res.results[4 * b + g]["yT"]
        # yT tiles indexed s*N_DK+m, each [128 dcols, 512 tokens]
        yT = acc.reshape(NS, N_DK, 128, 512).transpose(1, 2, 0, 3).reshape(D, L)
        out[b] = yT.T
    return out
